# revision 18
# baseline (speedup 1.0000x reference)
"""Trainium2 Bass kernel for nn_ContrastiveCRFLoss.

Reference computation (per batch b, for N sampled pixels):
    sel_g = guidance[b, :, r, c]            # (Cg, N)
    sel_c = clusters[b, :, r, c]            # (K, N)
    cd[a,b'] = ||p_a - p_b'||^2             # coords
    gd[a,b'] = ||g_a - g_b'||^2
    sim = W1*exp(-cd/(2a) - gd/(2B)) + W2*exp(-cd/(2G))
    out = -(sel_c^T sel_c) * sim            # (N, N)

Strategy (pure data parallel, 4 batches per core on 8 cores):
  * Host packs each core's guidance+clusters shard pixel-major into a
    (H*W, 128) feature matrix; 32-col block per batch (27 clusters +
    3 guidance + 2 pad).
  * Device computes flat pixel offsets from coords, gathers the N sampled
    feature rows via indirect DMA (one 512B row per sample), builds
    augmented exp-argument rows in sample-major layout (free-dim slicing
    dodges the SBUF quarter-partition alignment rule), then PE-transposes
    to (channel, sample) staging for the matmuls.
  * exp arguments are separable quadratics computed directly by matmuls
    with augmented rows. Precision/speed interplay on the PE:
      - fp32 matmul: 4 cyc/row. fp32r: 1 cyc/row but operands and
        products are rounded to 12 significand bits.
      - fp32r products are EXACT when one operand is a small constant or
        both operands are <=12-bit integers -> all coordinate terms
        (integers <= 2^17, split into 12-bit hi/lo rows) go through an
        fp32r matmul at full accuracy, arranged as an aligned collapsing
        oct so big partial sums cancel before small rows join.
      - real-valued rows (guidance, norms, logs) go through a SECOND
        bf16 matmul (1 cyc/row) accumulating into the same PSUM bank:
        every value is split into 8-bit-exact bf16 pieces, so products
        are exact in the f32 accumulator (~f32 accuracy overall).
    E1 (per batch): fp32r K=8 oct + bf16 K=20.  E2 (coords only,
    batch-independent): fp32r K=10.  S: fp32r K=27 (1.6e-4 relative
    product rounding on a multiplicative term - negligible).
  * out tile = (-S) * (exp(E1) + exp(E2)), exp on ACT, add split between
    DVE and GpSimd, final multiply on DVE, DMA out on both HWDGE rings.
"""

import math

import numpy as np

# problem shape (hardcoded per contest contract)
B, CG, K, H, W = 32, 3, 27, 256, 256
N = 1000
N_CORES = 8
BPC = B // N_CORES  # batches per core
HW = H * W
CPB = 32  # feature cols per batch block (27 clusters + 3 guidance + 2 pad)
PADCH = 128

ALPHA, BETA, GAMMA = 0.5, 0.15, 0.05
W1, W2 = 10.0, 3.0

CHUNK = 125  # sample chunk (output tile rows)
NCHUNK = N // CHUNK  # 8
BSPLIT = 512  # output tile col split (psum bank)
E2W = 16  # narrow workspace cols per chunk (E2 coord rows)

GB = 1.0 / (2.0 * BETA)  # guidance distance coefficient
SGB = math.sqrt(2.0 * GB)  # guidance pre-scale so rows are plain splits


def _split_bits(x, keep):
    x = np.float32(x)
    mask = np.uint32(0xFFFFFFFF) << np.uint32(24 - keep)
    hi = np.float32((x.view(np.uint32) & mask).view(np.float32))
    return float(hi), float(x - hi)


LW1HI, LW1LO = _split_bits(math.log(W1), 8)   # bf16 rows
LW2HI, LW2LO = _split_bits(math.log(W2), 12)  # fp32r rows

_CACHE = {}


def _build():
    if "nc" in _CACHE:
        return _CACHE["nc"]

    import concourse.bacc as bacc
    import concourse.bass as bass
    import concourse.mybir as mybir
    import concourse.tile as tile
    from concourse.masks import make_identity

    f32 = mybir.dt.float32
    f32r = mybir.dt.float32r
    bf16 = mybir.dt.bfloat16
    i32 = mybir.dt.int32
    AF = mybir.ActivationFunctionType
    OP = mybir.AluOpType
    AX = mybir.AxisListType

    nc = bacc.Bacc("TRN2", target_bir_lowering=False, debug=False,
                   num_devices=N_CORES)
    feat = nc.dram_tensor("feat", [HW, PADCH], f32, kind="ExternalInput").ap()
    coords = nc.dram_tensor("coords", [2, N], i32, kind="ExternalInput").ap()
    out = nc.dram_tensor("out", [BPC, N, N], f32, kind="ExternalOutput").ap()

    def quarters(t, lo, hi):
        # (CHUNK, 8*128) workspace viewed as (p, chunk, quarter, col-slice)
        return t[:].rearrange("p (k q c) -> p k q c", q=4, c=CPB)[:, :, :, lo:hi]

    def e2c(t, lo, hi):
        # (CHUNK, 8*E2W) workspace viewed as (p, chunk, col-slice)
        return t[:].rearrange("p (k c) -> p k c", c=E2W)[:, :, lo:hi]

    with tile.TileContext(nc) as tc:
        with tc.tile_pool(name="pp", bufs=1) as pp, \
             tc.tile_pool(name="wp", bufs=3) as wp:
            # ---- persistent tiles ----
            ident = pp.tile([128, 128], f32)
            make_identity(nc, ident[:])
            rc_all = pp.tile([CHUNK, 2 * NCHUNK], i32)
            rcf_all = pp.tile([CHUNK, 2 * NCHUNK], f32)
            off_all = pp.tile([CHUNK, NCHUNK], i32)
            gath = pp.tile([CHUNK, NCHUNK * PADCH], f32)
            augL = pp.tile([CHUNK, NCHUNK * PADCH], f32)
            augR = pp.tile([CHUNK, NCHUNK * PADCH], f32)
            usq = pp.tile([CHUNK, NCHUNK * PADCH], f32)
            aug2L = pp.tile([CHUNK, NCHUNK * E2W], f32)
            aug2R = pp.tile([CHUNK, NCHUNK * E2W], f32)
            # staging: fp32r for integer-exact rows, bf16 for real rows
            F0 = pp.tile([64, N], f32r)
            F1 = pp.tile([64, N], f32r)
            cn0 = pp.tile([64, N], f32r)
            cn1 = pp.tile([64, N], f32r)
            m1b_l0 = pp.tile([64, N], bf16)
            m1b_l1 = pp.tile([64, N], bf16)
            m1b_r0 = pp.tile([64, N], bf16)
            m1b_r1 = pp.tile([64, N], bf16)
            m2l = pp.tile([10, N], f32r)
            m2r = pp.tile([10, N], f32r)

            # ---- coords column-major load + offsets ----
            for ci in range(NCHUNK):
                nc.sync.dma_start(
                    out=rc_all[:, 2 * ci:2 * ci + 2],
                    in_=coords[:, ci * CHUNK:(ci + 1) * CHUNK].rearrange(
                        "c n -> n c"),
                )
            rview = rc_all[:].rearrange("p (k c) -> p k c", c=2)
            nc.vector.tensor_scalar(
                out=off_all[:].unsqueeze(2),
                in0=rview[:, :, 0:1], scalar1=W, scalar2=None, op0=OP.mult)
            nc.vector.tensor_tensor(
                out=off_all[:].unsqueeze(2),
                in0=off_all[:].unsqueeze(2),
                in1=rview[:, :, 1:2], op=OP.add)
            nc.vector.tensor_copy(out=rcf_all[:], in_=rc_all[:])

            # ---- gather sampled feature rows ----
            for ci in range(NCHUNK):
                nc.gpsimd.indirect_dma_start(
                    out=gath[:, ci * PADCH:(ci + 1) * PADCH],
                    out_offset=None,
                    in_=feat[:],
                    in_offset=bass.IndirectOffsetOnAxis(
                        ap=off_all[:, ci:ci + 1], axis=0),
                )

            # ---- build aug workspaces (sample-major) ----
            # E1 is ONE bf16 matmul per batch (K=28). All coordinate values
            # are integers packed into 8-bit-exact bf16 pieces (Veltkamp at
            # s=16 leaves an integer remainder <= 256, itself bf16-exact),
            # arranged as an aligned oct that collapses to -cd before the
            # small real rows join the accumulation tree. Real rows are
            # 8-bit piece-split so their products are exact too.
            # E1 quarter slot map in augL/augR (L / R), K=28:
            #    0: r / r      1: r / r     2: c / c     3: c / c
            #    4: -nahi / 1  5: -nalo / 1 6: 1 / -nbhi 7: 1 / -nblo
            #    8+4j..11+4j (channel j, v = sqrt(2GB)*g split into 8-bit
            #      vh+vl): L [vh vh vl vl], R [vh vl vh vl]
            #   20-22: -(uh,um,ul) / 1   23-25: 1 / -(uh,um,ul)
            #   26: LW1HI / 1   27: LW1LO / 1
            #   with nc = r^2+c^2 (int), ugn = |v|^2/2 = GB*|g|^2.
            # E2 narrow workspace (16 cols/chunk), K=10, fp32r (12-bit
            # hi/lo split of nc, scales on the constant rows, aligned oct):
            #    0: 4r / 5r     1: 4c / 5c
            #    2: nchi / -10  3: nclo / -10
            #    4: 10 / -nchi  5: 10 / -nclo   6,7: 0 / 0
            #    8: LW2HI / 1   9: LW2LO / 1
            nc.gpsimd.memset(augL[:], 0.0)
            nc.gpsimd.memset(augR[:], 0.0)
            nc.gpsimd.memset(aug2L[:], 0.0)
            nc.gpsimd.memset(aug2R[:], 0.0)
            rsq = pp.tile([CHUNK, 2 * NCHUNK], f32)
            ncs = pp.tile([CHUNK, NCHUNK], f32)
            nct = pp.tile([CHUNK, 2 * NCHUNK], f32)  # 12-bit hi/lo (E2)
            nbt = pp.tile([CHUNK, 2 * NCHUNK], f32)  # 8-bit hi/lo (E1)
            nc.vector.tensor_tensor(out=rsq[:], in0=rcf_all[:], in1=rcf_all[:],
                                    op=OP.mult)
            nc.vector.reduce_sum(
                out=ncs[:].unsqueeze(2),
                in_=rsq[:].rearrange("p (k c) -> p k c", c=2), axis=AX.X)
            # Veltkamp split of nc at 12 bits for E2 (rsq as scratch)
            nc.vector.tensor_scalar_mul(rsq[:, 0:NCHUNK], ncs[:], 4097.0)
            nc.vector.tensor_tensor(out=rsq[:, NCHUNK:], in0=rsq[:, 0:NCHUNK],
                                    in1=ncs[:], op=OP.subtract)
            nc.vector.tensor_tensor(out=nct[:, 0:NCHUNK], in0=rsq[:, 0:NCHUNK],
                                    in1=rsq[:, NCHUNK:], op=OP.subtract)
            nc.vector.tensor_tensor(out=nct[:, NCHUNK:], in0=ncs[:],
                                    in1=nct[:, 0:NCHUNK], op=OP.subtract)
            # Veltkamp split of nc at 8 bits for E1
            nc.vector.tensor_scalar_mul(rsq[:, 0:NCHUNK], ncs[:], 65537.0)
            nc.vector.tensor_tensor(out=rsq[:, NCHUNK:], in0=rsq[:, 0:NCHUNK],
                                    in1=ncs[:], op=OP.subtract)
            nc.vector.tensor_tensor(out=nbt[:, 0:NCHUNK], in0=rsq[:, 0:NCHUNK],
                                    in1=rsq[:, NCHUNK:], op=OP.subtract)
            nc.vector.tensor_tensor(out=nbt[:, NCHUNK:], in0=ncs[:],
                                    in1=nbt[:, 0:NCHUNK], op=OP.subtract)
            rcf_b = rcf_all[:].rearrange("p (k c) -> p k c", c=2) \
                .unsqueeze(2).broadcast_to([CHUNK, NCHUNK, 4, 2])
            rcf_v = rcf_all[:].rearrange("p (k c) -> p k c", c=2)
            nct_v = nct[:].rearrange("p (j k) -> p k j", j=2)
            nbt_b = nbt[:].rearrange("p (j k) -> p k j", j=2) \
                .unsqueeze(2).broadcast_to([CHUNK, NCHUNK, 4, 2])
            G = quarters(gath, K, K + CG)

            def fill_const(view, c):
                nc.vector.tensor_scalar(out=view, in0=view, scalar1=0.0,
                                        scalar2=c, op0=OP.mult, op1=OP.add)

            # v = sqrt(2GB)*g, split at 8 bits into usq cols 0-2 (vh),
            # 3-5 (vl); scratch 6-11; ugn in 12, pieces in 13-15.
            nc.vector.tensor_scalar_mul(quarters(usq, 6, 9), G, SGB)
            V = quarters(usq, 6, 9)
            nc.vector.tensor_scalar_mul(quarters(usq, 9, 12), V, 65537.0)
            nc.vector.tensor_tensor(out=quarters(usq, 0, 3),
                                    in0=quarters(usq, 9, 12), in1=V,
                                    op=OP.subtract)
            nc.vector.tensor_tensor(out=quarters(usq, 0, 3),
                                    in0=quarters(usq, 9, 12),
                                    in1=quarters(usq, 0, 3), op=OP.subtract)
            nc.vector.tensor_tensor(out=quarters(usq, 3, 6), in0=V,
                                    in1=quarters(usq, 0, 3), op=OP.subtract)
            # ugn = |v|^2 / 2
            nc.vector.tensor_tensor(out=quarters(usq, 9, 12), in0=V, in1=V,
                                    op=OP.mult)
            nc.vector.reduce_sum(out=quarters(usq, 12, 13),
                                 in_=quarters(usq, 9, 12), axis=AX.X)
            nc.vector.tensor_scalar_mul(quarters(usq, 12, 13),
                                        quarters(usq, 12, 13), 0.5)
            # 3-piece 8-bit split of ugn: uh(13) um(14) ul(15), scratch 6,7
            U = quarters(usq, 12, 13)
            nc.vector.tensor_scalar_mul(quarters(usq, 6, 7), U, 65537.0)
            nc.vector.tensor_tensor(out=quarters(usq, 13, 14),
                                    in0=quarters(usq, 6, 7), in1=U,
                                    op=OP.subtract)
            nc.vector.tensor_tensor(out=quarters(usq, 13, 14),
                                    in0=quarters(usq, 6, 7),
                                    in1=quarters(usq, 13, 14), op=OP.subtract)
            nc.vector.tensor_tensor(out=quarters(usq, 7, 8), in0=U,
                                    in1=quarters(usq, 13, 14), op=OP.subtract)
            R1 = quarters(usq, 7, 8)
            nc.vector.tensor_scalar_mul(quarters(usq, 6, 7), R1, 65537.0)
            nc.vector.tensor_tensor(out=quarters(usq, 14, 15),
                                    in0=quarters(usq, 6, 7), in1=R1,
                                    op=OP.subtract)
            nc.vector.tensor_tensor(out=quarters(usq, 14, 15),
                                    in0=quarters(usq, 6, 7),
                                    in1=quarters(usq, 14, 15), op=OP.subtract)
            nc.vector.tensor_tensor(out=quarters(usq, 15, 16), in0=R1,
                                    in1=quarters(usq, 14, 15), op=OP.subtract)

            # ---- E1 bf16 rows ----
            nc.vector.tensor_scalar_mul(quarters(augL, 0, 2),
                                        rcf_b[:, :, :, 0:1]
                                        .broadcast_to([CHUNK, NCHUNK, 4, 2]),
                                        1.0)
            nc.vector.tensor_scalar_mul(quarters(augL, 2, 4),
                                        rcf_b[:, :, :, 1:2]
                                        .broadcast_to([CHUNK, NCHUNK, 4, 2]),
                                        1.0)
            nc.vector.tensor_scalar_mul(quarters(augR, 0, 2),
                                        rcf_b[:, :, :, 0:1]
                                        .broadcast_to([CHUNK, NCHUNK, 4, 2]),
                                        1.0)
            nc.vector.tensor_scalar_mul(quarters(augR, 2, 4),
                                        rcf_b[:, :, :, 1:2]
                                        .broadcast_to([CHUNK, NCHUNK, 4, 2]),
                                        1.0)
            nc.vector.tensor_scalar_mul(quarters(augL, 4, 6), nbt_b, -1.0)
            fill_const(quarters(augL, 6, 8), 1.0)
            fill_const(quarters(augR, 4, 6), 1.0)
            nc.vector.tensor_scalar_mul(quarters(augR, 6, 8), nbt_b, -1.0)
            for j in range(CG):
                vhj = quarters(usq, j, j + 1).broadcast_to(
                    [CHUNK, NCHUNK, 4, 2])
                vlj = quarters(usq, CG + j, CG + j + 1).broadcast_to(
                    [CHUNK, NCHUNK, 4, 2])
                c0 = 8 + 4 * j
                nc.vector.tensor_scalar_mul(
                    quarters(augL, c0, c0 + 2), vhj, 1.0)
                nc.vector.tensor_scalar_mul(
                    quarters(augL, c0 + 2, c0 + 4), vlj, 1.0)
                rq = quarters(augR, c0, c0 + 4).rearrange(
                    "p k q (c h) -> p k q c h", h=2)
                nc.vector.tensor_scalar_mul(
                    rq[:, :, :, :, 0:1], vhj.unsqueeze(4), 1.0)
                nc.vector.tensor_scalar_mul(
                    rq[:, :, :, :, 1:2], vlj.unsqueeze(4), 1.0)
            nc.vector.tensor_scalar_mul(quarters(augL, 20, 23),
                                        quarters(usq, 13, 16), -1.0)
            fill_const(quarters(augL, 23, 26), 1.0)
            fill_const(quarters(augL, 26, 27), LW1HI)
            fill_const(quarters(augL, 27, 28), LW1LO)
            fill_const(quarters(augR, 20, 23), 1.0)
            nc.vector.tensor_scalar_mul(quarters(augR, 23, 26),
                                        quarters(usq, 13, 16), -1.0)
            fill_const(quarters(augR, 26, 28), 1.0)
            # ---- E2 narrow workspace (fp32r) ----
            nc.vector.tensor_scalar_mul(e2c(aug2L, 0, 2), rcf_v, 4.0)
            nc.vector.tensor_copy(out=e2c(aug2L, 2, 4), in_=nct_v)
            fill_const(e2c(aug2L, 4, 6), 10.0)
            fill_const(e2c(aug2L, 8, 9), LW2HI)
            fill_const(e2c(aug2L, 9, 10), LW2LO)
            nc.vector.tensor_scalar_mul(e2c(aug2R, 0, 2), rcf_v, 5.0)
            fill_const(e2c(aug2R, 2, 4), -10.0)
            nc.vector.tensor_scalar_mul(e2c(aug2R, 4, 6), nct_v, -1.0)
            fill_const(e2c(aug2R, 8, 10), 1.0)

            # ---- transpose to (channel, sample) staging ----
            with tc.tile_pool(name="ppsum", bufs=3, space="PSUM") as ppsum:
                for ci in range(NCHUNK):
                    sl = slice(ci * CHUNK, (ci + 1) * CHUNK)
                    csl = slice(ci * PADCH, (ci + 1) * PADCH)
                    ptF = ppsum.tile([128, CHUNK], f32, tag="trans")
                    nc.tensor.transpose(out=ptF[:], in_=gath[:, csl],
                                        identity=ident[0:CHUNK, 0:CHUNK])
                    nc.vector.tensor_copy(out=F0[:, sl], in_=ptF[0:64, :])
                    nc.vector.tensor_copy(out=F1[:, sl], in_=ptF[64:128, :])
                    nc.vector.tensor_scalar_mul(cn0[:, sl], ptF[0:64, :], -1.0)
                    nc.vector.tensor_scalar_mul(cn1[:, sl], ptF[64:128, :],
                                                -1.0)
                    ptL = ppsum.tile([128, CHUNK], f32, tag="trans")
                    nc.tensor.transpose(out=ptL[:], in_=augL[:, csl],
                                        identity=ident[0:CHUNK, 0:CHUNK])
                    nc.scalar.copy(out=m1b_l0[:, sl], in_=ptL[0:64, :])
                    nc.scalar.copy(out=m1b_l1[:, sl], in_=ptL[64:128, :])
                    ptR = ppsum.tile([128, CHUNK], f32, tag="trans")
                    nc.tensor.transpose(out=ptR[:], in_=augR[:, csl],
                                        identity=ident[0:CHUNK, 0:CHUNK])
                    nc.scalar.copy(out=m1b_r0[:, sl], in_=ptR[0:64, :])
                    nc.scalar.copy(out=m1b_r1[:, sl], in_=ptR[64:128, :])
                    ptEL = ppsum.tile([10, CHUNK], f32, tag="transE")
                    nc.tensor.transpose(
                        out=ptEL[:],
                        in_=aug2L[:, ci * E2W:ci * E2W + 10],
                        identity=ident[0:CHUNK, 0:CHUNK])
                    nc.scalar.copy(out=m2l[:, sl], in_=ptEL[:])
                    ptER = ppsum.tile([10, CHUNK], f32, tag="transE")
                    nc.tensor.transpose(
                        out=ptER[:],
                        in_=aug2R[:, ci * E2W:ci * E2W + 10],
                        identity=ident[0:CHUNK, 0:CHUNK])
                    nc.scalar.copy(out=m2r[:, sl], in_=ptER[:])

            # ---- main loop ----
            with tc.tile_pool(name="psS", bufs=2, space="PSUM") as psS, \
                 tc.tile_pool(name="psE", bufs=2, space="PSUM") as psE:
                for ai in range(NCHUNK):
                    asl = slice(ai * CHUNK, (ai + 1) * CHUNK)
                    pe2 = psE.tile([CHUNK, 1024], f32, tag="expmm")
                    nc.tensor.matmul(out=pe2[:, 0:BSPLIT], lhsT=m2l[:, asl],
                                     rhs=m2r[:, 0:BSPLIT], start=True, stop=True)
                    nc.tensor.matmul(out=pe2[:, BSPLIT:N], lhsT=m2l[:, asl],
                                     rhs=m2r[:, BSPLIT:N], start=True, stop=True)
                    e2sb = wp.tile([CHUNK, N], f32, tag="e2sb")
                    nc.scalar.activation(out=e2sb[:], in_=pe2[:, 0:N], func=AF.Exp)

                    for bi in range(BPC):
                        ps = psS.tile([CHUNK, 1024], f32, tag="smm")
                        Fb = F0 if bi < 2 else F1
                        Cb = cn0 if bi < 2 else cn1
                        Lb = m1b_l0 if bi < 2 else m1b_l1
                        Rb = m1b_r0 if bi < 2 else m1b_r1
                        qb = CPB * (bi % 2)
                        ck = slice(qb, qb + K)
                        bk = slice(qb, qb + 28)
                        nc.tensor.matmul(out=ps[:, 0:BSPLIT],
                                         lhsT=Cb[ck, asl], rhs=Fb[ck, 0:BSPLIT],
                                         start=True, stop=True)
                        nc.tensor.matmul(out=ps[:, BSPLIT:N],
                                         lhsT=Cb[ck, asl], rhs=Fb[ck, BSPLIT:N],
                                         start=True, stop=True)
                        pe1 = psE.tile([CHUNK, 1024], f32, tag="expmm")
                        nc.tensor.matmul(out=pe1[:, 0:BSPLIT],
                                         lhsT=Lb[bk, asl], rhs=Rb[bk, 0:BSPLIT],
                                         start=True, stop=True)
                        nc.tensor.matmul(out=pe1[:, BSPLIT:N],
                                         lhsT=Lb[bk, asl], rhs=Rb[bk, BSPLIT:N],
                                         start=True, stop=True)
                        e1sb = wp.tile([CHUNK, N], f32, tag="e1sb")
                        nc.scalar.activation(out=e1sb[:], in_=pe1[:, 0:N],
                                             func=AF.Exp)
                        simsb = wp.tile([CHUNK, N], f32, tag="simsb")
                        if bi % 2 == 0:
                            nc.gpsimd.tensor_tensor(out=simsb[:], in0=e1sb[:],
                                                    in1=e2sb[:], op=OP.add)
                        else:
                            nc.vector.tensor_tensor(out=simsb[:], in0=e1sb[:],
                                                    in1=e2sb[:], op=OP.add)
                        ressb = wp.tile([CHUNK, N], f32, tag="ressb")
                        nc.vector.tensor_tensor(out=ressb[:], in0=ps[:, 0:N],
                                                in1=simsb[:], op=OP.mult)
                        eng = nc.sync if bi % 2 == 0 else nc.scalar
                        eng.dma_start(out=out[bi, asl, :], in_=ressb[:])

    nc.compile()
    _CACHE["nc"] = nc
    return nc


def make_in_maps(guidance, clusters, coords):
    guidance = np.ascontiguousarray(guidance, dtype=np.float32)
    clusters = np.ascontiguousarray(clusters, dtype=np.float32)
    coords = np.ascontiguousarray(coords, dtype=np.int32)
    in_maps = []
    for c in range(N_CORES):
        b0 = c * BPC
        f = np.zeros((HW, PADCH), dtype=np.float32)
        for bi in range(BPC):
            f[:, CPB * bi:CPB * bi + K] = clusters[b0 + bi].reshape(K, HW).T
            f[:, CPB * bi + K:CPB * bi + K + CG] = (
                guidance[b0 + bi].reshape(CG, HW).T)
        in_maps.append({"feat": f, "coords": coords})
    return in_maps


def run_on_hw(in_maps, trace=False, **kw):
    from concourse.bass_utils import run_bass_kernel_spmd

    nc = _build()
    return run_bass_kernel_spmd(nc, in_maps, list(range(N_CORES)),
                                trace=trace, **kw)


def kernel(guidance, clusters, coords):
    res = run_on_hw(make_in_maps(guidance, clusters, coords))
    return np.concatenate([res.results[i]["out"] for i in range(N_CORES)],
                          axis=0)


# revision 19
# speedup vs baseline: 1.1585x; 1.1585x over previous
"""Trainium2 Bass kernel for nn_ContrastiveCRFLoss.

Reference computation (per batch b, for N sampled pixels):
    sel_g = guidance[b, :, r, c]            # (Cg, N)
    sel_c = clusters[b, :, r, c]            # (K, N)
    cd[a,b'] = ||p_a - p_b'||^2             # coords
    gd[a,b'] = ||g_a - g_b'||^2
    sim = W1*exp(-cd/(2a) - gd/(2B)) + W2*exp(-cd/(2G))
    out = -(sel_c^T sel_c) * sim            # (N, N)

Strategy (pure data parallel, 4 batches per core on 8 cores):
  * Host packs each core's guidance+clusters shard pixel-major into a
    (H*W, 128) feature matrix; 32-col block per batch (27 clusters +
    3 guidance + 2 pad).
  * Device computes flat pixel offsets from coords, gathers the N sampled
    feature rows via indirect DMA (one 512B row per sample), builds
    augmented exp-argument rows in sample-major layout (free-dim slicing
    dodges the SBUF quarter-partition alignment rule), then PE-transposes
    to (channel, sample) staging for the matmuls.
  * exp arguments are separable quadratics computed directly by matmuls
    with augmented rows. Precision/speed interplay on the PE:
      - fp32 matmul: 4 cyc/row. fp32r: 1 cyc/row but operands and
        products are rounded to 12 significand bits.
      - fp32r products are EXACT when one operand is a small constant or
        both operands are <=12-bit integers -> all coordinate terms
        (integers <= 2^17, split into 12-bit hi/lo rows) go through an
        fp32r matmul at full accuracy, arranged as an aligned collapsing
        oct so big partial sums cancel before small rows join.
      - real-valued rows (guidance, norms, logs) go through a SECOND
        bf16 matmul (1 cyc/row) accumulating into the same PSUM bank:
        every value is split into 8-bit-exact bf16 pieces, so products
        are exact in the f32 accumulator (~f32 accuracy overall).
    E1 (per batch): fp32r K=8 oct + bf16 K=20.  E2 (coords only,
    batch-independent): fp32r K=10.  S: fp32r K=27 (1.6e-4 relative
    product rounding on a multiplicative term - negligible).
  * out tile = (-S) * (exp(E1) + exp(E2)), exp on ACT, add split between
    DVE and GpSimd, final multiply on DVE, DMA out on both HWDGE rings.
"""

import math

import numpy as np

# problem shape (hardcoded per contest contract)
B, CG, K, H, W = 32, 3, 27, 256, 256
N = 1000
N_CORES = 8
BPC = B // N_CORES  # batches per core
HW = H * W
CPB = 32  # feature cols per batch block (27 clusters + 3 guidance + 2 pad)
PADCH = 128

ALPHA, BETA, GAMMA = 0.5, 0.15, 0.05
W1, W2 = 10.0, 3.0

CHUNK = 125  # sample chunk (output tile rows)
NCHUNK = N // CHUNK  # 8
BSPLIT = 512  # output tile col split (psum bank)
E2W = 16  # narrow workspace cols per chunk (E2 coord rows)

GB = 1.0 / (2.0 * BETA)  # guidance distance coefficient
SGB = math.sqrt(2.0 * GB)  # guidance pre-scale so rows are plain splits


def _split_bits(x, keep):
    x = np.float32(x)
    mask = np.uint32(0xFFFFFFFF) << np.uint32(24 - keep)
    hi = np.float32((x.view(np.uint32) & mask).view(np.float32))
    return float(hi), float(x - hi)


LW1HI, LW1LO = _split_bits(math.log(W1), 8)   # bf16 rows
LW2HI, LW2LO = _split_bits(math.log(W2), 12)  # fp32r rows

_CACHE = {}


def _build():
    if "nc" in _CACHE:
        return _CACHE["nc"]

    import concourse.bacc as bacc
    import concourse.bass as bass
    import concourse.mybir as mybir
    import concourse.tile as tile
    from concourse.masks import make_identity

    f32 = mybir.dt.float32
    f32r = mybir.dt.float32r
    bf16 = mybir.dt.bfloat16
    i32 = mybir.dt.int32
    AF = mybir.ActivationFunctionType
    OP = mybir.AluOpType
    AX = mybir.AxisListType

    nc = bacc.Bacc("TRN2", target_bir_lowering=False, debug=False,
                   num_devices=N_CORES)
    feat = nc.dram_tensor("feat", [HW, PADCH], f32, kind="ExternalInput").ap()
    coords = nc.dram_tensor("coords", [2, N], i32, kind="ExternalInput").ap()
    out = nc.dram_tensor("out", [BPC, N, N], f32, kind="ExternalOutput").ap()

    def quarters(t, lo, hi):
        # (CHUNK, 8*128) workspace viewed as (p, chunk, quarter, col-slice)
        return t[:].rearrange("p (k q c) -> p k q c", q=4, c=CPB)[:, :, :, lo:hi]

    def e2c(t, lo, hi):
        # (CHUNK, 8*E2W) workspace viewed as (p, chunk, col-slice)
        return t[:].rearrange("p (k c) -> p k c", c=E2W)[:, :, lo:hi]

    with tile.TileContext(nc) as tc:
        with tc.tile_pool(name="pp", bufs=1) as pp, \
             tc.tile_pool(name="wp", bufs=3) as wp:
            # ---- persistent tiles ----
            ident = pp.tile([128, 128], f32)
            make_identity(nc, ident[:])
            rc_all = pp.tile([CHUNK, 2 * NCHUNK], i32)
            rcf_all = pp.tile([CHUNK, 2 * NCHUNK], f32)
            off_all = pp.tile([CHUNK, NCHUNK], i32)
            gath = pp.tile([CHUNK, NCHUNK * PADCH], f32)
            augL = pp.tile([CHUNK, NCHUNK * PADCH], f32)
            augR = pp.tile([CHUNK, NCHUNK * PADCH], f32)
            usq = pp.tile([CHUNK, NCHUNK * PADCH], f32)
            aug2L = pp.tile([CHUNK, NCHUNK * E2W], f32)
            aug2R = pp.tile([CHUNK, NCHUNK * E2W], f32)
            # staging: fp32r for integer-exact rows, bf16 for real rows
            F0 = pp.tile([64, N], f32r)
            F1 = pp.tile([64, N], f32r)
            cn0 = pp.tile([64, N], f32r)
            cn1 = pp.tile([64, N], f32r)
            m1b_l0 = pp.tile([64, N], bf16)
            m1b_l1 = pp.tile([64, N], bf16)
            m1b_r0 = pp.tile([64, N], bf16)
            m1b_r1 = pp.tile([64, N], bf16)
            m2l = pp.tile([10, N], f32r)
            m2r = pp.tile([10, N], f32r)

            # ---- coords column-major load + offsets ----
            for ci in range(NCHUNK):
                nc.sync.dma_start(
                    out=rc_all[:, 2 * ci:2 * ci + 2],
                    in_=coords[:, ci * CHUNK:(ci + 1) * CHUNK].rearrange(
                        "c n -> n c"),
                )
            rview = rc_all[:].rearrange("p (k c) -> p k c", c=2)
            nc.vector.tensor_scalar(
                out=off_all[:].unsqueeze(2),
                in0=rview[:, :, 0:1], scalar1=W, scalar2=None, op0=OP.mult)
            nc.vector.tensor_tensor(
                out=off_all[:].unsqueeze(2),
                in0=off_all[:].unsqueeze(2),
                in1=rview[:, :, 1:2], op=OP.add)
            nc.vector.tensor_copy(out=rcf_all[:], in_=rc_all[:])

            # ---- gather sampled feature rows ----
            for ci in range(NCHUNK):
                nc.gpsimd.indirect_dma_start(
                    out=gath[:, ci * PADCH:(ci + 1) * PADCH],
                    out_offset=None,
                    in_=feat[:],
                    in_offset=bass.IndirectOffsetOnAxis(
                        ap=off_all[:, ci:ci + 1], axis=0),
                )

            # ---- build aug workspaces (sample-major) ----
            # E1 is ONE bf16 matmul per batch (K=28). All coordinate values
            # are integers packed into 8-bit-exact bf16 pieces (Veltkamp at
            # s=16 leaves an integer remainder <= 256, itself bf16-exact),
            # arranged as an aligned oct that collapses to -cd before the
            # small real rows join the accumulation tree. Real rows are
            # 8-bit piece-split so their products are exact too.
            # E1 quarter slot map in augL/augR (L / R), K=28:
            #    0: r / r      1: r / r     2: c / c     3: c / c
            #    4: -nahi / 1  5: -nalo / 1 6: 1 / -nbhi 7: 1 / -nblo
            #    8+4j..11+4j (channel j, v = sqrt(2GB)*g split into 8-bit
            #      vh+vl): L [vh vh vl vl], R [vh vl vh vl]
            #   20-22: -(uh,um,ul) / 1   23-25: 1 / -(uh,um,ul)
            #   26: LW1HI / 1   27: LW1LO / 1
            #   with nc = r^2+c^2 (int), ugn = |v|^2/2 = GB*|g|^2.
            # E2 narrow workspace (16 cols/chunk), K=10, fp32r (12-bit
            # hi/lo split of nc, scales on the constant rows, aligned oct):
            #    0: 4r / 5r     1: 4c / 5c
            #    2: nchi / -10  3: nclo / -10
            #    4: 10 / -nchi  5: 10 / -nclo   6,7: 0 / 0
            #    8: LW2HI / 1   9: LW2LO / 1
            nc.gpsimd.memset(augL[:], 0.0)
            nc.gpsimd.memset(augR[:], 0.0)
            nc.gpsimd.memset(aug2L[:], 0.0)
            nc.gpsimd.memset(aug2R[:], 0.0)
            rsq = pp.tile([CHUNK, 2 * NCHUNK], f32)
            ncs = pp.tile([CHUNK, NCHUNK], f32)
            nct = pp.tile([CHUNK, 2 * NCHUNK], f32)  # 12-bit hi/lo (E2)
            nbt = pp.tile([CHUNK, 2 * NCHUNK], f32)  # 8-bit hi/lo (E1)
            nc.vector.tensor_tensor(out=rsq[:], in0=rcf_all[:], in1=rcf_all[:],
                                    op=OP.mult)
            nc.vector.reduce_sum(
                out=ncs[:].unsqueeze(2),
                in_=rsq[:].rearrange("p (k c) -> p k c", c=2), axis=AX.X)
            # Veltkamp split of nc at 12 bits for E2 (rsq as scratch)
            nc.vector.tensor_scalar_mul(rsq[:, 0:NCHUNK], ncs[:], 4097.0)
            nc.vector.tensor_tensor(out=rsq[:, NCHUNK:], in0=rsq[:, 0:NCHUNK],
                                    in1=ncs[:], op=OP.subtract)
            nc.vector.tensor_tensor(out=nct[:, 0:NCHUNK], in0=rsq[:, 0:NCHUNK],
                                    in1=rsq[:, NCHUNK:], op=OP.subtract)
            nc.vector.tensor_tensor(out=nct[:, NCHUNK:], in0=ncs[:],
                                    in1=nct[:, 0:NCHUNK], op=OP.subtract)
            # Veltkamp split of nc at 8 bits for E1
            nc.vector.tensor_scalar_mul(rsq[:, 0:NCHUNK], ncs[:], 65537.0)
            nc.vector.tensor_tensor(out=rsq[:, NCHUNK:], in0=rsq[:, 0:NCHUNK],
                                    in1=ncs[:], op=OP.subtract)
            nc.vector.tensor_tensor(out=nbt[:, 0:NCHUNK], in0=rsq[:, 0:NCHUNK],
                                    in1=rsq[:, NCHUNK:], op=OP.subtract)
            nc.vector.tensor_tensor(out=nbt[:, NCHUNK:], in0=ncs[:],
                                    in1=nbt[:, 0:NCHUNK], op=OP.subtract)
            rcf_b = rcf_all[:].rearrange("p (k c) -> p k c", c=2) \
                .unsqueeze(2).broadcast_to([CHUNK, NCHUNK, 4, 2])
            rcf_v = rcf_all[:].rearrange("p (k c) -> p k c", c=2)
            nct_v = nct[:].rearrange("p (j k) -> p k j", j=2)
            nbt_b = nbt[:].rearrange("p (j k) -> p k j", j=2) \
                .unsqueeze(2).broadcast_to([CHUNK, NCHUNK, 4, 2])
            G = quarters(gath, K, K + CG)

            def fill_const(view, c):
                nc.vector.tensor_scalar(out=view, in0=view, scalar1=0.0,
                                        scalar2=c, op0=OP.mult, op1=OP.add)

            # v = sqrt(2GB)*g, split at 8 bits into usq cols 0-2 (vh),
            # 3-5 (vl); scratch 6-11; ugn in 12, pieces in 13-15.
            nc.vector.tensor_scalar_mul(quarters(usq, 6, 9), G, SGB)
            V = quarters(usq, 6, 9)
            nc.vector.tensor_scalar_mul(quarters(usq, 9, 12), V, 65537.0)
            nc.vector.tensor_tensor(out=quarters(usq, 0, 3),
                                    in0=quarters(usq, 9, 12), in1=V,
                                    op=OP.subtract)
            nc.vector.tensor_tensor(out=quarters(usq, 0, 3),
                                    in0=quarters(usq, 9, 12),
                                    in1=quarters(usq, 0, 3), op=OP.subtract)
            nc.vector.tensor_tensor(out=quarters(usq, 3, 6), in0=V,
                                    in1=quarters(usq, 0, 3), op=OP.subtract)
            # ugn = |v|^2 / 2
            nc.vector.tensor_tensor(out=quarters(usq, 9, 12), in0=V, in1=V,
                                    op=OP.mult)
            nc.vector.reduce_sum(out=quarters(usq, 12, 13),
                                 in_=quarters(usq, 9, 12), axis=AX.X)
            nc.vector.tensor_scalar_mul(quarters(usq, 12, 13),
                                        quarters(usq, 12, 13), 0.5)
            # 3-piece 8-bit split of ugn: uh(13) um(14) ul(15), scratch 6,7
            U = quarters(usq, 12, 13)
            nc.vector.tensor_scalar_mul(quarters(usq, 6, 7), U, 65537.0)
            nc.vector.tensor_tensor(out=quarters(usq, 13, 14),
                                    in0=quarters(usq, 6, 7), in1=U,
                                    op=OP.subtract)
            nc.vector.tensor_tensor(out=quarters(usq, 13, 14),
                                    in0=quarters(usq, 6, 7),
                                    in1=quarters(usq, 13, 14), op=OP.subtract)
            nc.vector.tensor_tensor(out=quarters(usq, 7, 8), in0=U,
                                    in1=quarters(usq, 13, 14), op=OP.subtract)
            R1 = quarters(usq, 7, 8)
            nc.vector.tensor_scalar_mul(quarters(usq, 6, 7), R1, 65537.0)
            nc.vector.tensor_tensor(out=quarters(usq, 14, 15),
                                    in0=quarters(usq, 6, 7), in1=R1,
                                    op=OP.subtract)
            nc.vector.tensor_tensor(out=quarters(usq, 14, 15),
                                    in0=quarters(usq, 6, 7),
                                    in1=quarters(usq, 14, 15), op=OP.subtract)
            nc.vector.tensor_tensor(out=quarters(usq, 15, 16), in0=R1,
                                    in1=quarters(usq, 14, 15), op=OP.subtract)

            # ---- E1 bf16 rows ----
            nc.vector.tensor_scalar_mul(quarters(augL, 0, 2),
                                        rcf_b[:, :, :, 0:1]
                                        .broadcast_to([CHUNK, NCHUNK, 4, 2]),
                                        1.0)
            nc.vector.tensor_scalar_mul(quarters(augL, 2, 4),
                                        rcf_b[:, :, :, 1:2]
                                        .broadcast_to([CHUNK, NCHUNK, 4, 2]),
                                        1.0)
            nc.vector.tensor_scalar_mul(quarters(augR, 0, 2),
                                        rcf_b[:, :, :, 0:1]
                                        .broadcast_to([CHUNK, NCHUNK, 4, 2]),
                                        1.0)
            nc.vector.tensor_scalar_mul(quarters(augR, 2, 4),
                                        rcf_b[:, :, :, 1:2]
                                        .broadcast_to([CHUNK, NCHUNK, 4, 2]),
                                        1.0)
            nc.vector.tensor_scalar_mul(quarters(augL, 4, 6), nbt_b, -1.0)
            fill_const(quarters(augL, 6, 8), 1.0)
            fill_const(quarters(augR, 4, 6), 1.0)
            nc.vector.tensor_scalar_mul(quarters(augR, 6, 8), nbt_b, -1.0)
            for j in range(CG):
                vhj = quarters(usq, j, j + 1).broadcast_to(
                    [CHUNK, NCHUNK, 4, 2])
                vlj = quarters(usq, CG + j, CG + j + 1).broadcast_to(
                    [CHUNK, NCHUNK, 4, 2])
                c0 = 8 + 4 * j
                nc.vector.tensor_scalar_mul(
                    quarters(augL, c0, c0 + 2), vhj, 1.0)
                nc.vector.tensor_scalar_mul(
                    quarters(augL, c0 + 2, c0 + 4), vlj, 1.0)
                rq = quarters(augR, c0, c0 + 4).rearrange(
                    "p k q (c h) -> p k q c h", h=2)
                nc.vector.tensor_scalar_mul(
                    rq[:, :, :, :, 0:1], vhj.unsqueeze(4), 1.0)
                nc.vector.tensor_scalar_mul(
                    rq[:, :, :, :, 1:2], vlj.unsqueeze(4), 1.0)
            nc.vector.tensor_scalar_mul(quarters(augL, 20, 23),
                                        quarters(usq, 13, 16), -1.0)
            fill_const(quarters(augL, 23, 26), 1.0)
            fill_const(quarters(augL, 26, 27), LW1HI)
            fill_const(quarters(augL, 27, 28), LW1LO)
            fill_const(quarters(augR, 20, 23), 1.0)
            nc.vector.tensor_scalar_mul(quarters(augR, 23, 26),
                                        quarters(usq, 13, 16), -1.0)
            fill_const(quarters(augR, 26, 28), 1.0)
            # ---- E2 narrow workspace (fp32r) ----
            nc.vector.tensor_scalar_mul(e2c(aug2L, 0, 2), rcf_v, 4.0)
            nc.vector.tensor_copy(out=e2c(aug2L, 2, 4), in_=nct_v)
            fill_const(e2c(aug2L, 4, 6), 10.0)
            fill_const(e2c(aug2L, 8, 9), LW2HI)
            fill_const(e2c(aug2L, 9, 10), LW2LO)
            nc.vector.tensor_scalar_mul(e2c(aug2R, 0, 2), rcf_v, 5.0)
            fill_const(e2c(aug2R, 2, 4), -10.0)
            nc.vector.tensor_scalar_mul(e2c(aug2R, 4, 6), nct_v, -1.0)
            fill_const(e2c(aug2R, 8, 10), 1.0)

            # ---- transpose to (channel, sample) staging ----
            with tc.tile_pool(name="ppsum", bufs=3, space="PSUM") as ppsum:
                for ci in range(NCHUNK):
                    sl = slice(ci * CHUNK, (ci + 1) * CHUNK)
                    csl = slice(ci * PADCH, (ci + 1) * PADCH)
                    ptF = ppsum.tile([128, CHUNK], f32, tag="trans")
                    nc.tensor.transpose(out=ptF[:], in_=gath[:, csl],
                                        identity=ident[0:CHUNK, 0:CHUNK])
                    nc.vector.tensor_copy(out=F0[:, sl], in_=ptF[0:64, :])
                    nc.vector.tensor_copy(out=F1[:, sl], in_=ptF[64:128, :])
                    nc.vector.tensor_scalar_mul(cn0[:, sl], ptF[0:64, :], -1.0)
                    nc.vector.tensor_scalar_mul(cn1[:, sl], ptF[64:128, :],
                                                -1.0)
                    ptL = ppsum.tile([128, CHUNK], f32, tag="trans")
                    nc.tensor.transpose(out=ptL[:], in_=augL[:, csl],
                                        identity=ident[0:CHUNK, 0:CHUNK])
                    nc.scalar.copy(out=m1b_l0[:, sl], in_=ptL[0:64, :])
                    nc.scalar.copy(out=m1b_l1[:, sl], in_=ptL[64:128, :])
                    ptR = ppsum.tile([128, CHUNK], f32, tag="trans")
                    nc.tensor.transpose(out=ptR[:], in_=augR[:, csl],
                                        identity=ident[0:CHUNK, 0:CHUNK])
                    nc.scalar.copy(out=m1b_r0[:, sl], in_=ptR[0:64, :])
                    nc.scalar.copy(out=m1b_r1[:, sl], in_=ptR[64:128, :])
                    ptEL = ppsum.tile([10, CHUNK], f32, tag="transE")
                    nc.tensor.transpose(
                        out=ptEL[:],
                        in_=aug2L[:, ci * E2W:ci * E2W + 10],
                        identity=ident[0:CHUNK, 0:CHUNK])
                    nc.scalar.copy(out=m2l[:, sl], in_=ptEL[:])
                    ptER = ppsum.tile([10, CHUNK], f32, tag="transE")
                    nc.tensor.transpose(
                        out=ptER[:],
                        in_=aug2R[:, ci * E2W:ci * E2W + 10],
                        identity=ident[0:CHUNK, 0:CHUNK])
                    nc.scalar.copy(out=m2r[:, sl], in_=ptER[:])

            # ---- main loop ----
            with tc.tile_pool(name="psS", bufs=2, space="PSUM") as psS, \
                 tc.tile_pool(name="psE", bufs=2, space="PSUM") as psE:
                for ai in range(NCHUNK):
                    asl = slice(ai * CHUNK, (ai + 1) * CHUNK)
                    pe2 = psE.tile([CHUNK, 1024], f32, tag="expmm")
                    nc.tensor.matmul(out=pe2[:, 0:BSPLIT], lhsT=m2l[:, asl],
                                     rhs=m2r[:, 0:BSPLIT], start=True, stop=True)
                    nc.tensor.matmul(out=pe2[:, BSPLIT:N], lhsT=m2l[:, asl],
                                     rhs=m2r[:, BSPLIT:N], start=True, stop=True)
                    e2sb = wp.tile([CHUNK, N], f32, tag="e2sb")
                    nc.scalar.activation(out=e2sb[:], in_=pe2[:, 0:N], func=AF.Exp)

                    for bi in range(BPC):
                        ps = psS.tile([CHUNK, 1024], f32, tag="smm")
                        Fb = F0 if bi < 2 else F1
                        Cb = cn0 if bi < 2 else cn1
                        Lb = m1b_l0 if bi < 2 else m1b_l1
                        Rb = m1b_r0 if bi < 2 else m1b_r1
                        qb = CPB * (bi % 2)
                        ck = slice(qb, qb + K)
                        bk = slice(qb, qb + 28)
                        nc.tensor.matmul(out=ps[:, 0:BSPLIT],
                                         lhsT=Cb[ck, asl], rhs=Fb[ck, 0:BSPLIT],
                                         start=True, stop=True)
                        nc.tensor.matmul(out=ps[:, BSPLIT:N],
                                         lhsT=Cb[ck, asl], rhs=Fb[ck, BSPLIT:N],
                                         start=True, stop=True)
                        pe1 = psE.tile([CHUNK, 1024], f32, tag="expmm")
                        nc.tensor.matmul(out=pe1[:, 0:BSPLIT],
                                         lhsT=Lb[bk, asl], rhs=Rb[bk, 0:BSPLIT],
                                         start=True, stop=True)
                        nc.tensor.matmul(out=pe1[:, BSPLIT:N],
                                         lhsT=Lb[bk, asl], rhs=Rb[bk, BSPLIT:N],
                                         start=True, stop=True)
                        e1sb = wp.tile([CHUNK, N], f32, tag="e1sb")
                        nc.scalar.activation(out=e1sb[:], in_=pe1[:, 0:N],
                                             func=AF.Exp)
                        simsb = wp.tile([CHUNK, N], f32, tag="simsb")
                        if bi < 2:
                            nc.gpsimd.tensor_tensor(out=simsb[:], in0=e1sb[:],
                                                    in1=e2sb[:], op=OP.add)
                        else:
                            nc.vector.tensor_tensor(out=simsb[:], in0=e1sb[:],
                                                    in1=e2sb[:], op=OP.add)
                        if bi == 0:
                            # HWDGE ring (SDMA engines 0-4)
                            ressb = wp.tile([CHUNK, N], f32, tag="ressb")
                            nc.vector.tensor_tensor(out=ressb[:],
                                                    in0=ps[:, 0:N],
                                                    in1=simsb[:], op=OP.mult)
                            nc.sync.dma_start(out=out[0, asl, :], in_=ressb[:])
                        else:
                            # batches 1-3 pack into one SWDGE DMA
                            # (SDMA engines 5-15)
                            if bi == 1:
                                res3 = wp.tile([CHUNK, 3 * N], f32, tag="res3")
                            nc.vector.tensor_tensor(
                                out=res3[:, (bi - 1) * N:bi * N],
                                in0=ps[:, 0:N], in1=simsb[:], op=OP.mult)
                            if bi == 3:
                                nc.gpsimd.dma_start(
                                    out=out[1:4, asl, :].rearrange(
                                        "b a c -> a b c"),
                                    in_=res3[:].rearrange(
                                        "a (b c) -> a b c", b=3))

    nc.compile()
    _CACHE["nc"] = nc
    return nc


def make_in_maps(guidance, clusters, coords):
    guidance = np.ascontiguousarray(guidance, dtype=np.float32)
    clusters = np.ascontiguousarray(clusters, dtype=np.float32)
    coords = np.ascontiguousarray(coords, dtype=np.int32)
    in_maps = []
    for c in range(N_CORES):
        b0 = c * BPC
        f = np.zeros((HW, PADCH), dtype=np.float32)
        for bi in range(BPC):
            f[:, CPB * bi:CPB * bi + K] = clusters[b0 + bi].reshape(K, HW).T
            f[:, CPB * bi + K:CPB * bi + K + CG] = (
                guidance[b0 + bi].reshape(CG, HW).T)
        in_maps.append({"feat": f, "coords": coords})
    return in_maps


def run_on_hw(in_maps, trace=False, **kw):
    from concourse.bass_utils import run_bass_kernel_spmd

    nc = _build()
    return run_bass_kernel_spmd(nc, in_maps, list(range(N_CORES)),
                                trace=trace, **kw)


def kernel(guidance, clusters, coords):
    res = run_on_hw(make_in_maps(guidance, clusters, coords))
    return np.concatenate([res.results[i]["out"] for i in range(N_CORES)],
                          axis=0)


# revision 21
# speedup vs baseline: 1.2182x; 1.0515x over previous
"""Trainium2 Bass kernel for nn_ContrastiveCRFLoss.

Reference computation (per batch b, for N sampled pixels):
    sel_g = guidance[b, :, r, c]            # (Cg, N)
    sel_c = clusters[b, :, r, c]            # (K, N)
    cd[a,b'] = ||p_a - p_b'||^2             # coords
    gd[a,b'] = ||g_a - g_b'||^2
    sim = W1*exp(-cd/(2a) - gd/(2B)) + W2*exp(-cd/(2G))
    out = -(sel_c^T sel_c) * sim            # (N, N)

Strategy (pure data parallel, 4 batches per core on 8 cores):
  * Host packs each core's guidance+clusters shard pixel-major into a
    (H*W, 128) feature matrix; 32-col block per batch (27 clusters +
    3 guidance + 2 pad).
  * Device computes flat pixel offsets from coords, gathers the N sampled
    feature rows via indirect DMA (one 512B row per sample), builds
    augmented exp-argument rows in sample-major layout (free-dim slicing
    dodges the SBUF quarter-partition alignment rule), then PE-transposes
    to (channel, sample) staging for the matmuls.
  * exp arguments are separable quadratics computed directly by matmuls
    with augmented rows. Precision/speed interplay on the PE:
      - fp32 matmul: 4 cyc/row. fp32r: 1 cyc/row but operands and
        products are rounded to 12 significand bits.
      - fp32r products are EXACT when one operand is a small constant or
        both operands are <=12-bit integers -> all coordinate terms
        (integers <= 2^17, split into 12-bit hi/lo rows) go through an
        fp32r matmul at full accuracy, arranged as an aligned collapsing
        oct so big partial sums cancel before small rows join.
      - real-valued rows (guidance, norms, logs) go through a SECOND
        bf16 matmul (1 cyc/row) accumulating into the same PSUM bank:
        every value is split into 8-bit-exact bf16 pieces, so products
        are exact in the f32 accumulator (~f32 accuracy overall).
    E1 (per batch): fp32r K=8 oct + bf16 K=20.  E2 (coords only,
    batch-independent): fp32r K=10.  S: fp32r K=27 (1.6e-4 relative
    product rounding on a multiplicative term - negligible).
  * out tile = (-S) * (exp(E1) + exp(E2)), exp on ACT, add split between
    DVE and GpSimd, final multiply on DVE, DMA out on both HWDGE rings.
"""

import math

import numpy as np

# problem shape (hardcoded per contest contract)
B, CG, K, H, W = 32, 3, 27, 256, 256
N = 1000
N_CORES = 8
BPC = B // N_CORES  # batches per core
HW = H * W
CPB = 32  # feature cols per batch block (27 clusters + 3 guidance + 2 pad)
PADCH = 128

ALPHA, BETA, GAMMA = 0.5, 0.15, 0.05
W1, W2 = 10.0, 3.0

CHUNK = 125  # sample chunk (output tile rows)
NCHUNK = N // CHUNK  # 8
BSPLIT = 512  # output tile col split (psum bank)
E2W = 16  # narrow workspace cols per chunk (E2 coord rows)

GB = 1.0 / (2.0 * BETA)  # guidance distance coefficient
SGB = math.sqrt(2.0 * GB)  # guidance pre-scale so rows are plain splits


def _split_bits(x, keep):
    x = np.float32(x)
    mask = np.uint32(0xFFFFFFFF) << np.uint32(24 - keep)
    hi = np.float32((x.view(np.uint32) & mask).view(np.float32))
    return float(hi), float(x - hi)


LW1HI, LW1LO = _split_bits(math.log(W1), 8)   # bf16 rows
LW2HI, LW2LO = _split_bits(math.log(W2), 12)  # fp32r rows

_CACHE = {}


def _build():
    if "nc" in _CACHE:
        return _CACHE["nc"]

    import concourse.bacc as bacc
    import concourse.bass as bass
    import concourse.mybir as mybir
    import concourse.tile as tile
    from concourse.masks import make_identity

    f32 = mybir.dt.float32
    f32r = mybir.dt.float32r
    bf16 = mybir.dt.bfloat16
    i32 = mybir.dt.int32
    AF = mybir.ActivationFunctionType
    OP = mybir.AluOpType
    AX = mybir.AxisListType

    nc = bacc.Bacc("TRN2", target_bir_lowering=False, debug=False,
                   num_devices=N_CORES)
    feat = nc.dram_tensor("feat", [HW, PADCH], f32, kind="ExternalInput").ap()
    coords = nc.dram_tensor("coords", [2, N], i32, kind="ExternalInput").ap()
    out = nc.dram_tensor("out", [BPC, N, N], f32, kind="ExternalOutput").ap()

    def quarters(t, lo, hi):
        # (CHUNK, 8*128) workspace viewed as (p, chunk, quarter, col-slice)
        return t[:].rearrange("p (k q c) -> p k q c", q=4, c=CPB)[:, :, :, lo:hi]

    def e2c(t, lo, hi):
        # (CHUNK, 8*E2W) workspace viewed as (p, chunk, col-slice)
        return t[:].rearrange("p (k c) -> p k c", c=E2W)[:, :, lo:hi]

    with tile.TileContext(nc) as tc:
        with tc.tile_pool(name="pp", bufs=1) as pp, \
             tc.tile_pool(name="wp", bufs=4) as wp:
            # ---- persistent tiles ----
            ident = pp.tile([128, 128], f32)
            make_identity(nc, ident[:])
            rc_all = pp.tile([CHUNK, 2 * NCHUNK], i32)
            rcf_all = pp.tile([CHUNK, 2 * NCHUNK], f32)
            off_all = pp.tile([CHUNK, NCHUNK], i32)
            gath = pp.tile([CHUNK, NCHUNK * PADCH], f32)
            augL = pp.tile([CHUNK, NCHUNK * PADCH], f32)
            augR = pp.tile([CHUNK, NCHUNK * PADCH], f32)
            usq = pp.tile([CHUNK, NCHUNK * PADCH], f32)
            aug2L = pp.tile([CHUNK, NCHUNK * E2W], f32)
            aug2R = pp.tile([CHUNK, NCHUNK * E2W], f32)
            # staging: fp32r for integer-exact rows, bf16 for real rows
            F0 = pp.tile([64, N], f32r)
            F1 = pp.tile([64, N], f32r)
            cn0 = pp.tile([64, N], f32r)
            cn1 = pp.tile([64, N], f32r)
            m1b_l0 = pp.tile([64, N], bf16)
            m1b_l1 = pp.tile([64, N], bf16)
            m1b_r0 = pp.tile([64, N], bf16)
            m1b_r1 = pp.tile([64, N], bf16)
            m2l = pp.tile([10, N], f32r)
            m2r = pp.tile([10, N], f32r)

            # ---- coords column-major load + offsets ----
            for ci in range(NCHUNK):
                nc.sync.dma_start(
                    out=rc_all[:, 2 * ci:2 * ci + 2],
                    in_=coords[:, ci * CHUNK:(ci + 1) * CHUNK].rearrange(
                        "c n -> n c"),
                )
            rview = rc_all[:].rearrange("p (k c) -> p k c", c=2)
            nc.vector.tensor_scalar(
                out=off_all[:].unsqueeze(2),
                in0=rview[:, :, 0:1], scalar1=W, scalar2=None, op0=OP.mult)
            nc.vector.tensor_tensor(
                out=off_all[:].unsqueeze(2),
                in0=off_all[:].unsqueeze(2),
                in1=rview[:, :, 1:2], op=OP.add)
            nc.vector.tensor_copy(out=rcf_all[:], in_=rc_all[:])

            # ---- gather sampled feature rows ----
            for ci in range(NCHUNK):
                nc.gpsimd.indirect_dma_start(
                    out=gath[:, ci * PADCH:(ci + 1) * PADCH],
                    out_offset=None,
                    in_=feat[:],
                    in_offset=bass.IndirectOffsetOnAxis(
                        ap=off_all[:, ci:ci + 1], axis=0),
                )

            # ---- build aug workspaces (sample-major) ----
            # E1 is ONE bf16 matmul per batch (K=28). All coordinate values
            # are integers packed into 8-bit-exact bf16 pieces (Veltkamp at
            # s=16 leaves an integer remainder <= 256, itself bf16-exact),
            # arranged as an aligned oct that collapses to -cd before the
            # small real rows join the accumulation tree. Real rows are
            # 8-bit piece-split so their products are exact too.
            # E1 quarter slot map in augL/augR (L / R), K=28:
            #    0: r / r      1: r / r     2: c / c     3: c / c
            #    4: -nahi / 1  5: -nalo / 1 6: 1 / -nbhi 7: 1 / -nblo
            #    8+4j..11+4j (channel j, v = sqrt(2GB)*g split into 8-bit
            #      vh+vl): L [vh vh vl vl], R [vh vl vh vl]
            #   20-22: -(uh,um,ul) / 1   23-25: 1 / -(uh,um,ul)
            #   26: LW1HI / 1   27: LW1LO / 1
            #   with nc = r^2+c^2 (int), ugn = |v|^2/2 = GB*|g|^2.
            # E2 narrow workspace (16 cols/chunk), K=10, fp32r (12-bit
            # hi/lo split of nc, scales on the constant rows, aligned oct):
            #    0: 4r / 5r     1: 4c / 5c
            #    2: nchi / -10  3: nclo / -10
            #    4: 10 / -nchi  5: 10 / -nclo   6,7: 0 / 0
            #    8: LW2HI / 1   9: LW2LO / 1
            nc.gpsimd.memset(augL[:], 0.0)
            nc.gpsimd.memset(augR[:], 0.0)
            nc.gpsimd.memset(aug2L[:], 0.0)
            nc.gpsimd.memset(aug2R[:], 0.0)
            rsq = pp.tile([CHUNK, 2 * NCHUNK], f32)
            ncs = pp.tile([CHUNK, NCHUNK], f32)
            nct = pp.tile([CHUNK, 2 * NCHUNK], f32)  # 12-bit hi/lo (E2)
            nbt = pp.tile([CHUNK, 2 * NCHUNK], f32)  # 8-bit hi/lo (E1)
            nc.vector.tensor_tensor(out=rsq[:], in0=rcf_all[:], in1=rcf_all[:],
                                    op=OP.mult)
            nc.vector.reduce_sum(
                out=ncs[:].unsqueeze(2),
                in_=rsq[:].rearrange("p (k c) -> p k c", c=2), axis=AX.X)
            # Veltkamp split of nc at 12 bits for E2 (rsq as scratch)
            nc.vector.tensor_scalar_mul(rsq[:, 0:NCHUNK], ncs[:], 4097.0)
            nc.vector.tensor_tensor(out=rsq[:, NCHUNK:], in0=rsq[:, 0:NCHUNK],
                                    in1=ncs[:], op=OP.subtract)
            nc.vector.tensor_tensor(out=nct[:, 0:NCHUNK], in0=rsq[:, 0:NCHUNK],
                                    in1=rsq[:, NCHUNK:], op=OP.subtract)
            nc.vector.tensor_tensor(out=nct[:, NCHUNK:], in0=ncs[:],
                                    in1=nct[:, 0:NCHUNK], op=OP.subtract)
            # Veltkamp split of nc at 8 bits for E1
            nc.vector.tensor_scalar_mul(rsq[:, 0:NCHUNK], ncs[:], 65537.0)
            nc.vector.tensor_tensor(out=rsq[:, NCHUNK:], in0=rsq[:, 0:NCHUNK],
                                    in1=ncs[:], op=OP.subtract)
            nc.vector.tensor_tensor(out=nbt[:, 0:NCHUNK], in0=rsq[:, 0:NCHUNK],
                                    in1=rsq[:, NCHUNK:], op=OP.subtract)
            nc.vector.tensor_tensor(out=nbt[:, NCHUNK:], in0=ncs[:],
                                    in1=nbt[:, 0:NCHUNK], op=OP.subtract)
            rcf_b = rcf_all[:].rearrange("p (k c) -> p k c", c=2) \
                .unsqueeze(2).broadcast_to([CHUNK, NCHUNK, 4, 2])
            rcf_v = rcf_all[:].rearrange("p (k c) -> p k c", c=2)
            nct_v = nct[:].rearrange("p (j k) -> p k j", j=2)
            nbt_b = nbt[:].rearrange("p (j k) -> p k j", j=2) \
                .unsqueeze(2).broadcast_to([CHUNK, NCHUNK, 4, 2])
            G = quarters(gath, K, K + CG)

            def fill_const(view, c):
                nc.vector.tensor_scalar(out=view, in0=view, scalar1=0.0,
                                        scalar2=c, op0=OP.mult, op1=OP.add)

            # v = sqrt(2GB)*g, split at 8 bits into usq cols 0-2 (vh),
            # 3-5 (vl); scratch 6-11; ugn in 12, pieces in 13-15.
            nc.vector.tensor_scalar_mul(quarters(usq, 6, 9), G, SGB)
            V = quarters(usq, 6, 9)
            nc.vector.tensor_scalar_mul(quarters(usq, 9, 12), V, 65537.0)
            nc.vector.tensor_tensor(out=quarters(usq, 0, 3),
                                    in0=quarters(usq, 9, 12), in1=V,
                                    op=OP.subtract)
            nc.vector.tensor_tensor(out=quarters(usq, 0, 3),
                                    in0=quarters(usq, 9, 12),
                                    in1=quarters(usq, 0, 3), op=OP.subtract)
            nc.vector.tensor_tensor(out=quarters(usq, 3, 6), in0=V,
                                    in1=quarters(usq, 0, 3), op=OP.subtract)
            # ugn = |v|^2 / 2
            nc.vector.tensor_tensor(out=quarters(usq, 9, 12), in0=V, in1=V,
                                    op=OP.mult)
            nc.vector.reduce_sum(out=quarters(usq, 12, 13),
                                 in_=quarters(usq, 9, 12), axis=AX.X)
            nc.vector.tensor_scalar_mul(quarters(usq, 12, 13),
                                        quarters(usq, 12, 13), 0.5)
            # 3-piece 8-bit split of ugn: uh(13) um(14) ul(15), scratch 6,7
            U = quarters(usq, 12, 13)
            nc.vector.tensor_scalar_mul(quarters(usq, 6, 7), U, 65537.0)
            nc.vector.tensor_tensor(out=quarters(usq, 13, 14),
                                    in0=quarters(usq, 6, 7), in1=U,
                                    op=OP.subtract)
            nc.vector.tensor_tensor(out=quarters(usq, 13, 14),
                                    in0=quarters(usq, 6, 7),
                                    in1=quarters(usq, 13, 14), op=OP.subtract)
            nc.vector.tensor_tensor(out=quarters(usq, 7, 8), in0=U,
                                    in1=quarters(usq, 13, 14), op=OP.subtract)
            R1 = quarters(usq, 7, 8)
            nc.vector.tensor_scalar_mul(quarters(usq, 6, 7), R1, 65537.0)
            nc.vector.tensor_tensor(out=quarters(usq, 14, 15),
                                    in0=quarters(usq, 6, 7), in1=R1,
                                    op=OP.subtract)
            nc.vector.tensor_tensor(out=quarters(usq, 14, 15),
                                    in0=quarters(usq, 6, 7),
                                    in1=quarters(usq, 14, 15), op=OP.subtract)
            nc.vector.tensor_tensor(out=quarters(usq, 15, 16), in0=R1,
                                    in1=quarters(usq, 14, 15), op=OP.subtract)

            # ---- E1 bf16 rows ----
            nc.vector.tensor_scalar_mul(quarters(augL, 0, 2),
                                        rcf_b[:, :, :, 0:1]
                                        .broadcast_to([CHUNK, NCHUNK, 4, 2]),
                                        1.0)
            nc.vector.tensor_scalar_mul(quarters(augL, 2, 4),
                                        rcf_b[:, :, :, 1:2]
                                        .broadcast_to([CHUNK, NCHUNK, 4, 2]),
                                        1.0)
            nc.vector.tensor_scalar_mul(quarters(augR, 0, 2),
                                        rcf_b[:, :, :, 0:1]
                                        .broadcast_to([CHUNK, NCHUNK, 4, 2]),
                                        1.0)
            nc.vector.tensor_scalar_mul(quarters(augR, 2, 4),
                                        rcf_b[:, :, :, 1:2]
                                        .broadcast_to([CHUNK, NCHUNK, 4, 2]),
                                        1.0)
            nc.vector.tensor_scalar_mul(quarters(augL, 4, 6), nbt_b, -1.0)
            fill_const(quarters(augL, 6, 8), 1.0)
            fill_const(quarters(augR, 4, 6), 1.0)
            nc.vector.tensor_scalar_mul(quarters(augR, 6, 8), nbt_b, -1.0)
            for j in range(CG):
                vhj = quarters(usq, j, j + 1).broadcast_to(
                    [CHUNK, NCHUNK, 4, 2])
                vlj = quarters(usq, CG + j, CG + j + 1).broadcast_to(
                    [CHUNK, NCHUNK, 4, 2])
                c0 = 8 + 4 * j
                nc.vector.tensor_scalar_mul(
                    quarters(augL, c0, c0 + 2), vhj, 1.0)
                nc.vector.tensor_scalar_mul(
                    quarters(augL, c0 + 2, c0 + 4), vlj, 1.0)
                rq = quarters(augR, c0, c0 + 4).rearrange(
                    "p k q (c h) -> p k q c h", h=2)
                nc.vector.tensor_scalar_mul(
                    rq[:, :, :, :, 0:1], vhj.unsqueeze(4), 1.0)
                nc.vector.tensor_scalar_mul(
                    rq[:, :, :, :, 1:2], vlj.unsqueeze(4), 1.0)
            nc.vector.tensor_scalar_mul(quarters(augL, 20, 23),
                                        quarters(usq, 13, 16), -1.0)
            fill_const(quarters(augL, 23, 26), 1.0)
            fill_const(quarters(augL, 26, 27), LW1HI)
            fill_const(quarters(augL, 27, 28), LW1LO)
            fill_const(quarters(augR, 20, 23), 1.0)
            nc.vector.tensor_scalar_mul(quarters(augR, 23, 26),
                                        quarters(usq, 13, 16), -1.0)
            fill_const(quarters(augR, 26, 28), 1.0)
            # ---- E2 narrow workspace (fp32r) ----
            nc.vector.tensor_scalar_mul(e2c(aug2L, 0, 2), rcf_v, 4.0)
            nc.vector.tensor_copy(out=e2c(aug2L, 2, 4), in_=nct_v)
            fill_const(e2c(aug2L, 4, 6), 10.0)
            fill_const(e2c(aug2L, 8, 9), LW2HI)
            fill_const(e2c(aug2L, 9, 10), LW2LO)
            nc.vector.tensor_scalar_mul(e2c(aug2R, 0, 2), rcf_v, 5.0)
            fill_const(e2c(aug2R, 2, 4), -10.0)
            nc.vector.tensor_scalar_mul(e2c(aug2R, 4, 6), nct_v, -1.0)
            fill_const(e2c(aug2R, 8, 10), 1.0)

            # ---- transpose to (channel, sample) staging ----
            with tc.tile_pool(name="ppsum", bufs=3, space="PSUM") as ppsum:
                for ci in range(NCHUNK):
                    sl = slice(ci * CHUNK, (ci + 1) * CHUNK)
                    csl = slice(ci * PADCH, (ci + 1) * PADCH)
                    ptF = ppsum.tile([128, CHUNK], f32, tag="trans")
                    nc.tensor.transpose(out=ptF[:], in_=gath[:, csl],
                                        identity=ident[0:CHUNK, 0:CHUNK])
                    nc.vector.tensor_copy(out=F0[:, sl], in_=ptF[0:64, :])
                    nc.vector.tensor_copy(out=F1[:, sl], in_=ptF[64:128, :])
                    nc.vector.tensor_scalar_mul(cn0[:, sl], ptF[0:64, :], -1.0)
                    nc.vector.tensor_scalar_mul(cn1[:, sl], ptF[64:128, :],
                                                -1.0)
                    ptL = ppsum.tile([128, CHUNK], f32, tag="trans")
                    nc.tensor.transpose(out=ptL[:], in_=augL[:, csl],
                                        identity=ident[0:CHUNK, 0:CHUNK])
                    nc.scalar.copy(out=m1b_l0[:, sl], in_=ptL[0:64, :])
                    nc.scalar.copy(out=m1b_l1[:, sl], in_=ptL[64:128, :])
                    ptR = ppsum.tile([128, CHUNK], f32, tag="trans")
                    nc.tensor.transpose(out=ptR[:], in_=augR[:, csl],
                                        identity=ident[0:CHUNK, 0:CHUNK])
                    nc.scalar.copy(out=m1b_r0[:, sl], in_=ptR[0:64, :])
                    nc.scalar.copy(out=m1b_r1[:, sl], in_=ptR[64:128, :])
                    ptEL = ppsum.tile([10, CHUNK], f32, tag="transE")
                    nc.tensor.transpose(
                        out=ptEL[:],
                        in_=aug2L[:, ci * E2W:ci * E2W + 10],
                        identity=ident[0:CHUNK, 0:CHUNK])
                    nc.scalar.copy(out=m2l[:, sl], in_=ptEL[:])
                    ptER = ppsum.tile([10, CHUNK], f32, tag="transE")
                    nc.tensor.transpose(
                        out=ptER[:],
                        in_=aug2R[:, ci * E2W:ci * E2W + 10],
                        identity=ident[0:CHUNK, 0:CHUNK])
                    nc.scalar.copy(out=m2r[:, sl], in_=ptER[:])

            # ---- main loop ----
            with tc.tile_pool(name="psS", bufs=2, space="PSUM") as psS, \
                 tc.tile_pool(name="psE", bufs=2, space="PSUM") as psE:
                for ai in range(NCHUNK):
                    asl = slice(ai * CHUNK, (ai + 1) * CHUNK)
                    pe2 = psE.tile([CHUNK, 1024], f32, tag="expmm")
                    nc.tensor.matmul(out=pe2[:, 0:BSPLIT], lhsT=m2l[:, asl],
                                     rhs=m2r[:, 0:BSPLIT], start=True, stop=True)
                    nc.tensor.matmul(out=pe2[:, BSPLIT:N], lhsT=m2l[:, asl],
                                     rhs=m2r[:, BSPLIT:N], start=True, stop=True)
                    e2sb = wp.tile([CHUNK, N], f32, tag="e2sb")
                    nc.scalar.activation(out=e2sb[:], in_=pe2[:, 0:N], func=AF.Exp)

                    for bi in range(BPC):
                        ps = psS.tile([CHUNK, 1024], f32, tag="smm")
                        Fb = F0 if bi < 2 else F1
                        Cb = cn0 if bi < 2 else cn1
                        Lb = m1b_l0 if bi < 2 else m1b_l1
                        Rb = m1b_r0 if bi < 2 else m1b_r1
                        qb = CPB * (bi % 2)
                        ck = slice(qb, qb + K)
                        bk = slice(qb, qb + 28)
                        nc.tensor.matmul(out=ps[:, 0:BSPLIT],
                                         lhsT=Cb[ck, asl], rhs=Fb[ck, 0:BSPLIT],
                                         start=True, stop=True)
                        nc.tensor.matmul(out=ps[:, BSPLIT:N],
                                         lhsT=Cb[ck, asl], rhs=Fb[ck, BSPLIT:N],
                                         start=True, stop=True)
                        pe1 = psE.tile([CHUNK, 1024], f32, tag="expmm")
                        nc.tensor.matmul(out=pe1[:, 0:BSPLIT],
                                         lhsT=Lb[bk, asl], rhs=Rb[bk, 0:BSPLIT],
                                         start=True, stop=True)
                        nc.tensor.matmul(out=pe1[:, BSPLIT:N],
                                         lhsT=Lb[bk, asl], rhs=Rb[bk, BSPLIT:N],
                                         start=True, stop=True)
                        e1sb = wp.tile([CHUNK, N], f32, tag="e1sb")
                        nc.scalar.activation(out=e1sb[:], in_=pe1[:, 0:N],
                                             func=AF.Exp)
                        simsb = wp.tile([CHUNK, N], f32, tag="simsb")
                        if bi % 2 == 0:
                            nc.gpsimd.tensor_tensor(out=simsb[:], in0=e1sb[:],
                                                    in1=e2sb[:], op=OP.add)
                        else:
                            nc.vector.tensor_tensor(out=simsb[:], in0=e1sb[:],
                                                    in1=e2sb[:], op=OP.add)
                        if bi == 0:
                            # HWDGE ring (SDMA engines 0-4)
                            ressb = wp.tile([CHUNK, N], f32, tag="ressb")
                            nc.vector.tensor_tensor(out=ressb[:],
                                                    in0=ps[:, 0:N],
                                                    in1=simsb[:], op=OP.mult)
                            nc.sync.dma_start(out=out[0, asl, :], in_=ressb[:])
                        else:
                            # batches 1-3 pack into one SWDGE DMA
                            # (SDMA engines 5-15)
                            if bi == 1:
                                res3 = wp.tile([CHUNK, 3 * N], f32, tag="res3")
                            nc.vector.tensor_tensor(
                                out=res3[:, (bi - 1) * N:bi * N],
                                in0=ps[:, 0:N], in1=simsb[:], op=OP.mult)
                            if bi == 3:
                                nc.gpsimd.dma_start(
                                    out=out[1:4, asl, :].rearrange(
                                        "b a c -> a b c"),
                                    in_=res3[:].rearrange(
                                        "a (b c) -> a b c", b=3))

    nc.compile()
    _CACHE["nc"] = nc
    return nc


def make_in_maps(guidance, clusters, coords):
    guidance = np.ascontiguousarray(guidance, dtype=np.float32)
    clusters = np.ascontiguousarray(clusters, dtype=np.float32)
    coords = np.ascontiguousarray(coords, dtype=np.int32)
    in_maps = []
    for c in range(N_CORES):
        b0 = c * BPC
        f = np.zeros((HW, PADCH), dtype=np.float32)
        for bi in range(BPC):
            f[:, CPB * bi:CPB * bi + K] = clusters[b0 + bi].reshape(K, HW).T
            f[:, CPB * bi + K:CPB * bi + K + CG] = (
                guidance[b0 + bi].reshape(CG, HW).T)
        in_maps.append({"feat": f, "coords": coords})
    return in_maps


def run_on_hw(in_maps, trace=False, **kw):
    from concourse.bass_utils import run_bass_kernel_spmd

    nc = _build()
    return run_bass_kernel_spmd(nc, in_maps, list(range(N_CORES)),
                                trace=trace, **kw)


def kernel(guidance, clusters, coords):
    res = run_on_hw(make_in_maps(guidance, clusters, coords))
    return np.concatenate([res.results[i]["out"] for i in range(N_CORES)],
                          axis=0)


# revision 22
# speedup vs baseline: 1.2969x; 1.0646x over previous
"""Trainium2 Bass kernel for nn_ContrastiveCRFLoss.

Reference computation (per batch b, for N sampled pixels):
    sel_g = guidance[b, :, r, c]            # (Cg, N)
    sel_c = clusters[b, :, r, c]            # (K, N)
    cd[a,b'] = ||p_a - p_b'||^2             # coords
    gd[a,b'] = ||g_a - g_b'||^2
    sim = W1*exp(-cd/(2a) - gd/(2B)) + W2*exp(-cd/(2G))
    out = -(sel_c^T sel_c) * sim            # (N, N)

Strategy (pure data parallel, 4 batches per core on 8 cores):
  * Host packs each core's guidance+clusters shard pixel-major into a
    (H*W, 128) feature matrix; 32-col block per batch (27 clusters +
    3 guidance + 2 pad).
  * Device computes flat pixel offsets from coords, gathers the N sampled
    feature rows via indirect DMA (one 512B row per sample), builds
    augmented exp-argument rows in sample-major layout (free-dim slicing
    dodges the SBUF quarter-partition alignment rule), then PE-transposes
    to (channel, sample) staging for the matmuls.
  * exp arguments are separable quadratics computed directly by matmuls
    with augmented rows. Precision/speed interplay on the PE:
      - fp32 matmul: 4 cyc/row. fp32r: 1 cyc/row but operands and
        products are rounded to 12 significand bits.
      - fp32r products are EXACT when one operand is a small constant or
        both operands are <=12-bit integers -> all coordinate terms
        (integers <= 2^17, split into 12-bit hi/lo rows) go through an
        fp32r matmul at full accuracy, arranged as an aligned collapsing
        oct so big partial sums cancel before small rows join.
      - real-valued rows (guidance, norms, logs) go through a SECOND
        bf16 matmul (1 cyc/row) accumulating into the same PSUM bank:
        every value is split into 8-bit-exact bf16 pieces, so products
        are exact in the f32 accumulator (~f32 accuracy overall).
    E1 (per batch): fp32r K=8 oct + bf16 K=20.  E2 (coords only,
    batch-independent): fp32r K=10.  S: fp32r K=27 (1.6e-4 relative
    product rounding on a multiplicative term - negligible).
  * out tile = (-S) * (exp(E1) + exp(E2)), exp on ACT, add split between
    DVE and GpSimd, final multiply on DVE, DMA out on both HWDGE rings.
"""

import math

import numpy as np

# problem shape (hardcoded per contest contract)
B, CG, K, H, W = 32, 3, 27, 256, 256
N = 1000
N_CORES = 8
BPC = B // N_CORES  # batches per core
HW = H * W
CPB = 32  # feature cols per batch block (27 clusters + 3 guidance + 2 pad)
PADCH = 128

ALPHA, BETA, GAMMA = 0.5, 0.15, 0.05
W1, W2 = 10.0, 3.0

CHUNK = 125  # sample chunk (output tile rows)
NCHUNK = N // CHUNK  # 8
BSPLIT = 512  # output tile col split (psum bank)
E2W = 16  # narrow workspace cols per chunk (E2 coord rows)

GB = 1.0 / (2.0 * BETA)  # guidance distance coefficient
SGB = math.sqrt(2.0 * GB)  # guidance pre-scale so rows are plain splits


def _split_bits(x, keep):
    x = np.float32(x)
    mask = np.uint32(0xFFFFFFFF) << np.uint32(24 - keep)
    hi = np.float32((x.view(np.uint32) & mask).view(np.float32))
    return float(hi), float(x - hi)


LW1HI, LW1LO = _split_bits(math.log(W1), 8)   # bf16 rows
LW2HI, LW2LO = _split_bits(math.log(W2), 12)  # fp32r rows

_CACHE = {}


def _build():
    if "nc" in _CACHE:
        return _CACHE["nc"]

    import concourse.bacc as bacc
    import concourse.bass as bass
    import concourse.mybir as mybir
    import concourse.tile as tile
    from concourse.masks import make_identity

    f32 = mybir.dt.float32
    f32r = mybir.dt.float32r
    bf16 = mybir.dt.bfloat16
    i32 = mybir.dt.int32
    AF = mybir.ActivationFunctionType
    OP = mybir.AluOpType
    AX = mybir.AxisListType

    nc = bacc.Bacc("TRN2", target_bir_lowering=False, debug=False,
                   num_devices=N_CORES)
    feat = nc.dram_tensor("feat", [HW, PADCH], f32, kind="ExternalInput").ap()
    coords = nc.dram_tensor("coords", [2, N], i32, kind="ExternalInput").ap()
    out = nc.dram_tensor("out", [BPC, N, N], f32, kind="ExternalOutput").ap()

    def quarters(t, lo, hi):
        # (CHUNK, 8*128) workspace viewed as (p, chunk, quarter, col-slice)
        return t[:].rearrange("p (k q c) -> p k q c", q=4, c=CPB)[:, :, :, lo:hi]

    def e2c(t, lo, hi):
        # (CHUNK, 8*E2W) workspace viewed as (p, chunk, col-slice)
        return t[:].rearrange("p (k c) -> p k c", c=E2W)[:, :, lo:hi]

    with tile.TileContext(nc) as tc:
        with tc.tile_pool(name="pp", bufs=1) as pp, \
             tc.tile_pool(name="wp", bufs=4) as wp:
            # ---- persistent tiles ----
            ident = pp.tile([128, 128], f32)
            make_identity(nc, ident[:])
            rc_all = pp.tile([CHUNK, 2 * NCHUNK], i32)
            rcf_all = pp.tile([CHUNK, 2 * NCHUNK], f32)
            off_all = pp.tile([CHUNK, NCHUNK], i32)
            gath = pp.tile([CHUNK, NCHUNK * PADCH], f32)
            augL = pp.tile([CHUNK, NCHUNK * PADCH], f32)
            augR = pp.tile([CHUNK, NCHUNK * PADCH], f32)
            usq = pp.tile([CHUNK, NCHUNK * PADCH], f32)
            aug2L = pp.tile([CHUNK, NCHUNK * E2W], f32)
            aug2R = pp.tile([CHUNK, NCHUNK * E2W], f32)
            # staging: fp32r for integer-exact rows, bf16 for real rows
            F0 = pp.tile([64, N], f32r)
            F1 = pp.tile([64, N], f32r)
            cn0 = pp.tile([64, N], f32r)
            cn1 = pp.tile([64, N], f32r)
            m1b_l0 = pp.tile([64, N], bf16)
            m1b_l1 = pp.tile([64, N], bf16)
            m1b_r0 = pp.tile([64, N], bf16)
            m1b_r1 = pp.tile([64, N], bf16)
            m2l = pp.tile([10, N], f32r)
            m2r = pp.tile([10, N], f32r)

            # ---- coords column-major load + offsets ----
            for ci in range(NCHUNK):
                nc.sync.dma_start(
                    out=rc_all[:, 2 * ci:2 * ci + 2],
                    in_=coords[:, ci * CHUNK:(ci + 1) * CHUNK].rearrange(
                        "c n -> n c"),
                )
            rview = rc_all[:].rearrange("p (k c) -> p k c", c=2)
            nc.vector.tensor_scalar(
                out=off_all[:].unsqueeze(2),
                in0=rview[:, :, 0:1], scalar1=W, scalar2=None, op0=OP.mult)
            nc.vector.tensor_tensor(
                out=off_all[:].unsqueeze(2),
                in0=off_all[:].unsqueeze(2),
                in1=rview[:, :, 1:2], op=OP.add)
            nc.vector.tensor_copy(out=rcf_all[:], in_=rc_all[:])

            # ---- gather sampled feature rows ----
            for ci in range(NCHUNK):
                nc.gpsimd.indirect_dma_start(
                    out=gath[:, ci * PADCH:(ci + 1) * PADCH],
                    out_offset=None,
                    in_=feat[:],
                    in_offset=bass.IndirectOffsetOnAxis(
                        ap=off_all[:, ci:ci + 1], axis=0),
                )

            # ---- build aug workspaces (sample-major) ----
            # E1 is ONE bf16 matmul per batch (K=28). All coordinate values
            # are integers packed into 8-bit-exact bf16 pieces (Veltkamp at
            # s=16 leaves an integer remainder <= 256, itself bf16-exact),
            # arranged as an aligned oct that collapses to -cd before the
            # small real rows join the accumulation tree. Real rows are
            # 8-bit piece-split so their products are exact too.
            # E1 quarter slot map in augL/augR (L / R), K=28:
            #    0: r / r      1: r / r     2: c / c     3: c / c
            #    4: -nahi / 1  5: -nalo / 1 6: 1 / -nbhi 7: 1 / -nblo
            #    8+4j..11+4j (channel j, v = sqrt(2GB)*g split into 8-bit
            #      vh+vl): L [vh vh vl vl], R [vh vl vh vl]
            #   20-22: -(uh,um,ul) / 1   23-25: 1 / -(uh,um,ul)
            #   26: LW1HI / 1   27: LW1LO / 1
            #   with nc = r^2+c^2 (int), ugn = |v|^2/2 = GB*|g|^2.
            # E2 narrow workspace (16 cols/chunk), K=10, fp32r (12-bit
            # hi/lo split of nc, scales on the constant rows, aligned oct):
            #    0: 4r / 5r     1: 4c / 5c
            #    2: nchi / -10  3: nclo / -10
            #    4: 10 / -nchi  5: 10 / -nclo   6,7: 0 / 0
            #    8: LW2HI / 1   9: LW2LO / 1
            nc.gpsimd.memset(augL[:], 0.0)
            nc.gpsimd.memset(augR[:], 0.0)
            nc.gpsimd.memset(aug2L[:], 0.0)
            nc.gpsimd.memset(aug2R[:], 0.0)
            rsq = pp.tile([CHUNK, 2 * NCHUNK], f32)
            ncs = pp.tile([CHUNK, NCHUNK], f32)
            nct = pp.tile([CHUNK, 2 * NCHUNK], f32)  # 12-bit hi/lo (E2)
            nbt = pp.tile([CHUNK, 2 * NCHUNK], f32)  # 8-bit hi/lo (E1)
            nc.vector.tensor_tensor(out=rsq[:], in0=rcf_all[:], in1=rcf_all[:],
                                    op=OP.mult)
            nc.vector.reduce_sum(
                out=ncs[:].unsqueeze(2),
                in_=rsq[:].rearrange("p (k c) -> p k c", c=2), axis=AX.X)
            # Veltkamp split of nc at 12 bits for E2 (rsq as scratch)
            nc.vector.tensor_scalar_mul(rsq[:, 0:NCHUNK], ncs[:], 4097.0)
            nc.vector.tensor_tensor(out=rsq[:, NCHUNK:], in0=rsq[:, 0:NCHUNK],
                                    in1=ncs[:], op=OP.subtract)
            nc.vector.tensor_tensor(out=nct[:, 0:NCHUNK], in0=rsq[:, 0:NCHUNK],
                                    in1=rsq[:, NCHUNK:], op=OP.subtract)
            nc.vector.tensor_tensor(out=nct[:, NCHUNK:], in0=ncs[:],
                                    in1=nct[:, 0:NCHUNK], op=OP.subtract)
            # Veltkamp split of nc at 8 bits for E1
            nc.vector.tensor_scalar_mul(rsq[:, 0:NCHUNK], ncs[:], 65537.0)
            nc.vector.tensor_tensor(out=rsq[:, NCHUNK:], in0=rsq[:, 0:NCHUNK],
                                    in1=ncs[:], op=OP.subtract)
            nc.vector.tensor_tensor(out=nbt[:, 0:NCHUNK], in0=rsq[:, 0:NCHUNK],
                                    in1=rsq[:, NCHUNK:], op=OP.subtract)
            nc.vector.tensor_tensor(out=nbt[:, NCHUNK:], in0=ncs[:],
                                    in1=nbt[:, 0:NCHUNK], op=OP.subtract)
            rcf_b = rcf_all[:].rearrange("p (k c) -> p k c", c=2) \
                .unsqueeze(2).broadcast_to([CHUNK, NCHUNK, 4, 2])
            rcf_v = rcf_all[:].rearrange("p (k c) -> p k c", c=2)
            nct_v = nct[:].rearrange("p (j k) -> p k j", j=2)
            nbt_b = nbt[:].rearrange("p (j k) -> p k j", j=2) \
                .unsqueeze(2).broadcast_to([CHUNK, NCHUNK, 4, 2])
            G = quarters(gath, K, K + CG)

            def fill_const(view, c):
                nc.vector.tensor_scalar(out=view, in0=view, scalar1=0.0,
                                        scalar2=c, op0=OP.mult, op1=OP.add)

            # v = sqrt(2GB)*g, split at 8 bits into usq cols 0-2 (vh),
            # 3-5 (vl); scratch 6-11; ugn in 12, pieces in 13-15.
            nc.vector.tensor_scalar_mul(quarters(usq, 6, 9), G, SGB)
            V = quarters(usq, 6, 9)
            nc.vector.tensor_scalar_mul(quarters(usq, 9, 12), V, 65537.0)
            nc.vector.tensor_tensor(out=quarters(usq, 0, 3),
                                    in0=quarters(usq, 9, 12), in1=V,
                                    op=OP.subtract)
            nc.vector.tensor_tensor(out=quarters(usq, 0, 3),
                                    in0=quarters(usq, 9, 12),
                                    in1=quarters(usq, 0, 3), op=OP.subtract)
            nc.vector.tensor_tensor(out=quarters(usq, 3, 6), in0=V,
                                    in1=quarters(usq, 0, 3), op=OP.subtract)
            # ugn = |v|^2 / 2
            nc.vector.tensor_tensor(out=quarters(usq, 9, 12), in0=V, in1=V,
                                    op=OP.mult)
            nc.vector.reduce_sum(out=quarters(usq, 12, 13),
                                 in_=quarters(usq, 9, 12), axis=AX.X)
            nc.vector.tensor_scalar_mul(quarters(usq, 12, 13),
                                        quarters(usq, 12, 13), 0.5)
            # 3-piece 8-bit split of ugn: uh(13) um(14) ul(15), scratch 6,7
            U = quarters(usq, 12, 13)
            nc.vector.tensor_scalar_mul(quarters(usq, 6, 7), U, 65537.0)
            nc.vector.tensor_tensor(out=quarters(usq, 13, 14),
                                    in0=quarters(usq, 6, 7), in1=U,
                                    op=OP.subtract)
            nc.vector.tensor_tensor(out=quarters(usq, 13, 14),
                                    in0=quarters(usq, 6, 7),
                                    in1=quarters(usq, 13, 14), op=OP.subtract)
            nc.vector.tensor_tensor(out=quarters(usq, 7, 8), in0=U,
                                    in1=quarters(usq, 13, 14), op=OP.subtract)
            R1 = quarters(usq, 7, 8)
            nc.vector.tensor_scalar_mul(quarters(usq, 6, 7), R1, 65537.0)
            nc.vector.tensor_tensor(out=quarters(usq, 14, 15),
                                    in0=quarters(usq, 6, 7), in1=R1,
                                    op=OP.subtract)
            nc.vector.tensor_tensor(out=quarters(usq, 14, 15),
                                    in0=quarters(usq, 6, 7),
                                    in1=quarters(usq, 14, 15), op=OP.subtract)
            nc.vector.tensor_tensor(out=quarters(usq, 15, 16), in0=R1,
                                    in1=quarters(usq, 14, 15), op=OP.subtract)

            # ---- E1 bf16 rows ----
            nc.vector.tensor_scalar_mul(quarters(augL, 0, 2),
                                        rcf_b[:, :, :, 0:1]
                                        .broadcast_to([CHUNK, NCHUNK, 4, 2]),
                                        1.0)
            nc.vector.tensor_scalar_mul(quarters(augL, 2, 4),
                                        rcf_b[:, :, :, 1:2]
                                        .broadcast_to([CHUNK, NCHUNK, 4, 2]),
                                        1.0)
            nc.vector.tensor_scalar_mul(quarters(augR, 0, 2),
                                        rcf_b[:, :, :, 0:1]
                                        .broadcast_to([CHUNK, NCHUNK, 4, 2]),
                                        1.0)
            nc.vector.tensor_scalar_mul(quarters(augR, 2, 4),
                                        rcf_b[:, :, :, 1:2]
                                        .broadcast_to([CHUNK, NCHUNK, 4, 2]),
                                        1.0)
            nc.vector.tensor_scalar_mul(quarters(augL, 4, 6), nbt_b, -1.0)
            fill_const(quarters(augL, 6, 8), 1.0)
            fill_const(quarters(augR, 4, 6), 1.0)
            nc.vector.tensor_scalar_mul(quarters(augR, 6, 8), nbt_b, -1.0)
            for j in range(CG):
                vhj = quarters(usq, j, j + 1).broadcast_to(
                    [CHUNK, NCHUNK, 4, 2])
                vlj = quarters(usq, CG + j, CG + j + 1).broadcast_to(
                    [CHUNK, NCHUNK, 4, 2])
                c0 = 8 + 4 * j
                nc.vector.tensor_scalar_mul(
                    quarters(augL, c0, c0 + 2), vhj, 1.0)
                nc.vector.tensor_scalar_mul(
                    quarters(augL, c0 + 2, c0 + 4), vlj, 1.0)
                rq = quarters(augR, c0, c0 + 4).rearrange(
                    "p k q (c h) -> p k q c h", h=2)
                nc.vector.tensor_scalar_mul(
                    rq[:, :, :, :, 0:1], vhj.unsqueeze(4), 1.0)
                nc.vector.tensor_scalar_mul(
                    rq[:, :, :, :, 1:2], vlj.unsqueeze(4), 1.0)
            nc.vector.tensor_scalar_mul(quarters(augL, 20, 23),
                                        quarters(usq, 13, 16), -1.0)
            fill_const(quarters(augL, 23, 26), 1.0)
            fill_const(quarters(augL, 26, 27), LW1HI)
            fill_const(quarters(augL, 27, 28), LW1LO)
            fill_const(quarters(augR, 20, 23), 1.0)
            nc.vector.tensor_scalar_mul(quarters(augR, 23, 26),
                                        quarters(usq, 13, 16), -1.0)
            fill_const(quarters(augR, 26, 28), 1.0)
            # ---- E2 narrow workspace (fp32r) ----
            nc.vector.tensor_scalar_mul(e2c(aug2L, 0, 2), rcf_v, 4.0)
            nc.vector.tensor_copy(out=e2c(aug2L, 2, 4), in_=nct_v)
            fill_const(e2c(aug2L, 4, 6), 10.0)
            fill_const(e2c(aug2L, 8, 9), LW2HI)
            fill_const(e2c(aug2L, 9, 10), LW2LO)
            nc.vector.tensor_scalar_mul(e2c(aug2R, 0, 2), rcf_v, 5.0)
            fill_const(e2c(aug2R, 2, 4), -10.0)
            nc.vector.tensor_scalar_mul(e2c(aug2R, 4, 6), nct_v, -1.0)
            fill_const(e2c(aug2R, 8, 10), 1.0)

            # ---- transpose to (channel, sample) staging ----
            with tc.tile_pool(name="ppsum", bufs=3, space="PSUM") as ppsum:
                for ci in range(NCHUNK):
                    sl = slice(ci * CHUNK, (ci + 1) * CHUNK)
                    csl = slice(ci * PADCH, (ci + 1) * PADCH)
                    ptF = ppsum.tile([128, CHUNK], f32, tag="trans")
                    nc.tensor.transpose(out=ptF[:], in_=gath[:, csl],
                                        identity=ident[0:CHUNK, 0:CHUNK])
                    nc.vector.tensor_copy(out=F0[:, sl], in_=ptF[0:64, :])
                    nc.vector.tensor_copy(out=F1[:, sl], in_=ptF[64:128, :])
                    nc.vector.tensor_scalar_mul(cn0[:, sl], ptF[0:64, :], -1.0)
                    nc.vector.tensor_scalar_mul(cn1[:, sl], ptF[64:128, :],
                                                -1.0)
                    ptL = ppsum.tile([128, CHUNK], f32, tag="trans")
                    nc.tensor.transpose(out=ptL[:], in_=augL[:, csl],
                                        identity=ident[0:CHUNK, 0:CHUNK])
                    nc.scalar.copy(out=m1b_l0[:, sl], in_=ptL[0:64, :])
                    nc.scalar.copy(out=m1b_l1[:, sl], in_=ptL[64:128, :])
                    ptR = ppsum.tile([128, CHUNK], f32, tag="trans")
                    nc.tensor.transpose(out=ptR[:], in_=augR[:, csl],
                                        identity=ident[0:CHUNK, 0:CHUNK])
                    nc.scalar.copy(out=m1b_r0[:, sl], in_=ptR[0:64, :])
                    nc.scalar.copy(out=m1b_r1[:, sl], in_=ptR[64:128, :])
                    ptEL = ppsum.tile([10, CHUNK], f32, tag="transE")
                    nc.tensor.transpose(
                        out=ptEL[:],
                        in_=aug2L[:, ci * E2W:ci * E2W + 10],
                        identity=ident[0:CHUNK, 0:CHUNK])
                    nc.scalar.copy(out=m2l[:, sl], in_=ptEL[:])
                    ptER = ppsum.tile([10, CHUNK], f32, tag="transE")
                    nc.tensor.transpose(
                        out=ptER[:],
                        in_=aug2R[:, ci * E2W:ci * E2W + 10],
                        identity=ident[0:CHUNK, 0:CHUNK])
                    nc.scalar.copy(out=m2r[:, sl], in_=ptER[:])

            # ---- main loop ----
            with tc.tile_pool(name="psS", bufs=2, space="PSUM") as psS, \
                 tc.tile_pool(name="psE", bufs=2, space="PSUM") as psE:
                for ai in range(NCHUNK):
                    asl = slice(ai * CHUNK, (ai + 1) * CHUNK)
                    pe2 = psE.tile([CHUNK, 1024], f32, tag="expmm")
                    nc.tensor.matmul(out=pe2[:, 0:BSPLIT], lhsT=m2l[:, asl],
                                     rhs=m2r[:, 0:BSPLIT], start=True, stop=True)
                    nc.tensor.matmul(out=pe2[:, BSPLIT:N], lhsT=m2l[:, asl],
                                     rhs=m2r[:, BSPLIT:N], start=True, stop=True)
                    e2sb = wp.tile([CHUNK, N], f32, tag="e2sb")
                    nc.scalar.activation(out=e2sb[:], in_=pe2[:, 0:N], func=AF.Exp)

                    for bi in range(BPC):
                        ps = psS.tile([CHUNK, 1024], f32, tag="smm")
                        Fb = F0 if bi < 2 else F1
                        Cb = cn0 if bi < 2 else cn1
                        Lb = m1b_l0 if bi < 2 else m1b_l1
                        Rb = m1b_r0 if bi < 2 else m1b_r1
                        qb = CPB * (bi % 2)
                        ck = slice(qb, qb + K)
                        bk = slice(qb, qb + 28)
                        nc.tensor.matmul(out=ps[:, 0:BSPLIT],
                                         lhsT=Cb[ck, asl], rhs=Fb[ck, 0:BSPLIT],
                                         start=True, stop=True)
                        nc.tensor.matmul(out=ps[:, BSPLIT:N],
                                         lhsT=Cb[ck, asl], rhs=Fb[ck, BSPLIT:N],
                                         start=True, stop=True)
                        pe1 = psE.tile([CHUNK, 1024], f32, tag="expmm")
                        nc.tensor.matmul(out=pe1[:, 0:BSPLIT],
                                         lhsT=Lb[bk, asl], rhs=Rb[bk, 0:BSPLIT],
                                         start=True, stop=True)
                        nc.tensor.matmul(out=pe1[:, BSPLIT:N],
                                         lhsT=Lb[bk, asl], rhs=Rb[bk, BSPLIT:N],
                                         start=True, stop=True)
                        simsb = wp.tile([CHUNK, N], f32, tag="simsb")
                        if bi % 2 == 0:
                            # SBUF add path on GpSimd
                            e1sb = wp.tile([CHUNK, N], f32, tag="e1sb")
                            nc.scalar.activation(out=e1sb[:], in_=pe1[:, 0:N],
                                                 func=AF.Exp)
                            nc.gpsimd.tensor_tensor(out=simsb[:], in0=e1sb[:],
                                                    in1=e2sb[:], op=OP.add)
                        else:
                            # in-place PSUM exp; PSUM-operand add on DVE
                            # (dodges SBUF port contention)
                            nc.scalar.activation(out=pe1[:, 0:N],
                                                 in_=pe1[:, 0:N], func=AF.Exp)
                            nc.vector.tensor_tensor(out=simsb[:],
                                                    in0=pe1[:, 0:N],
                                                    in1=e2sb[:], op=OP.add)
                        if bi == 0:
                            # HWDGE ring (SDMA engines 0-4)
                            ressb = wp.tile([CHUNK, N], f32, tag="ressb")
                            nc.vector.tensor_tensor(out=ressb[:],
                                                    in0=ps[:, 0:N],
                                                    in1=simsb[:], op=OP.mult)
                            nc.sync.dma_start(out=out[0, asl, :], in_=ressb[:])
                        else:
                            # batches 1-3 pack into one SWDGE DMA
                            # (SDMA engines 5-15)
                            if bi == 1:
                                res3 = wp.tile([CHUNK, 3 * N], f32, tag="res3")
                            nc.vector.tensor_tensor(
                                out=res3[:, (bi - 1) * N:bi * N],
                                in0=ps[:, 0:N], in1=simsb[:], op=OP.mult)
                            if bi == 3:
                                nc.gpsimd.dma_start(
                                    out=out[1:4, asl, :].rearrange(
                                        "b a c -> a b c"),
                                    in_=res3[:].rearrange(
                                        "a (b c) -> a b c", b=3))

    nc.compile()
    _CACHE["nc"] = nc
    return nc


def make_in_maps(guidance, clusters, coords):
    guidance = np.ascontiguousarray(guidance, dtype=np.float32)
    clusters = np.ascontiguousarray(clusters, dtype=np.float32)
    coords = np.ascontiguousarray(coords, dtype=np.int32)
    in_maps = []
    for c in range(N_CORES):
        b0 = c * BPC
        f = np.zeros((HW, PADCH), dtype=np.float32)
        for bi in range(BPC):
            f[:, CPB * bi:CPB * bi + K] = clusters[b0 + bi].reshape(K, HW).T
            f[:, CPB * bi + K:CPB * bi + K + CG] = (
                guidance[b0 + bi].reshape(CG, HW).T)
        in_maps.append({"feat": f, "coords": coords})
    return in_maps


def run_on_hw(in_maps, trace=False, **kw):
    from concourse.bass_utils import run_bass_kernel_spmd

    nc = _build()
    return run_bass_kernel_spmd(nc, in_maps, list(range(N_CORES)),
                                trace=trace, **kw)


def kernel(guidance, clusters, coords):
    res = run_on_hw(make_in_maps(guidance, clusters, coords))
    return np.concatenate([res.results[i]["out"] for i in range(N_CORES)],
                          axis=0)
